# revision 1
# baseline (speedup 1.0000x reference)
import numpy as np

f32 = np.float32
f64 = np.float64
B, N, K = 8, 4096, 16
LAST_EXEC_NS = None

try:
    from scipy.special import erf as _erf
except Exception:
    import math

    _erf = np.vectorize(math.erf, otypes=[np.float64])


def _fma32(a, b, c):
    return (a.astype(f64) * b.astype(f64) + c.astype(f64)).astype(f32)


def _gelu64(x):
    return 0.5 * x * (1.0 + _erf(x * 0.7071067811865476))


def _mlp(x, W1, b1, W2, b2):
    h = (x.astype(f64) @ W1.astype(f64) + b1.astype(f64)).astype(f32)
    h = _gelu64(h.astype(f64)).astype(f32)
    return (h.astype(f64) @ W2.astype(f64) + b2.astype(f64)).astype(f32)


def _batch_geometry(c):
    # bitwise-matching fp32 distance chain (plain sq, fma dot) + stable tiebreak
    x, y, z = c[:, 0], c[:, 1], c[:, 2]
    sq = ((x * x + y * y) + z * z).astype(f32)
    dot = _fma32(z[:, None], z[None, :],
                 _fma32(y[:, None], y[None, :], (x[:, None] * x[None, :]).astype(f32)))
    d = ((sq[:, None] + sq[None, :]).astype(f32) - (f32(2.0) * dot)).astype(f32)
    np.fill_diagonal(d, np.inf)
    # exact top-K by (value, index): 32-candidate partition + stable refine
    cand = np.argpartition(d, 32, axis=1)[:, :32]
    dc = np.take_along_axis(d, cand, axis=1)
    ordv = np.lexsort((cand, dc), axis=-1)[:, :K]
    idx = np.take_along_axis(cand, ordv, axis=1)

    rel = (c[idx] - c[:, None, :]).astype(f32)
    rel64 = rel.astype(f64)
    cov = np.einsum("nki,nkj->nij", rel64, rel64) / float(K)
    evals, evecs = np.linalg.eigh(cov)
    normals = evecs[..., 0]
    center = c.astype(f64).mean(axis=0).astype(f32)
    outward = (c - center).astype(f32)
    dt = (normals * outward.astype(f64)).sum(-1)
    orient = np.where(dt >= 0, 1.0, -1.0)
    normals = normals * orient[:, None]
    normals = normals / np.maximum(np.linalg.norm(normals, axis=-1, keepdims=True), 1e-6)
    normals = normals.astype(f32)

    radius = np.linalg.norm(rel64, axis=-1).mean(axis=-1).astype(f32)
    cr = np.linalg.norm(outward.astype(f64), axis=-1).astype(f32)
    ev32 = evals.astype(f32)
    eig_sum = np.maximum(ev32.sum(-1), f32(1e-6)).astype(f32)
    dominance = (ev32[:, 2] / eig_sum).astype(f32)
    invariants = np.concatenate(
        [ev32, radius[:, None], cr[:, None], dominance[:, None]], axis=-1
    ).astype(f32)
    return normals, invariants


def _compute_batch(c, feat, w):
    normals, invariants = _batch_geometry(c)
    inv_h = _mlp(invariants, w["inv_W1"], w["inv_b1"], w["inv_W2"], w["inv_b2"])
    feat_h = _mlp(feat, w["feat_W1"], w["feat_b1"], w["feat_W2"], w["feat_b2"])
    hidden = _mlp(np.concatenate([inv_h, feat_h], -1),
                  w["sh_W1"], w["sh_b1"], w["sh_W2"], w["sh_b2"])
    scalar = (hidden.astype(f64) @ np.asarray(w["g0_W"], f64)
              + np.asarray(w["g0_b"], f64)).astype(f32)
    o = np.zeros((N, 16), f32)
    o[:, 0:1] = scalar
    o[:, 1] = normals[:, 0]
    o[:, 2] = normals[:, 1]
    o[:, 3] = -normals[:, 2]
    o[:, 4] = -(c * normals).sum(-1)
    o[:, 5:8] = normals
    o[:, 11:14] = c
    o[:, 14] = 1.0
    return o


def _host_compute(inp):
    coords = np.asarray(inp["coords"], f32)
    feats = np.asarray(inp["features"], f32)
    w = {k: np.asarray(v) for k, v in inp.items() if k not in ("coords", "features")}
    outs = [_compute_batch(coords[b], feats[b], w) for b in range(B)]
    return np.stack(outs, axis=0)


def _bass_stage(host_out, trace=False):
    # SPMD device stage: each core streams its batch's [4096,16] output
    # through the NeuronCore (DRAM->DRAM DMA), data-parallel over B.
    global LAST_EXEC_NS
    import concourse.bass as bass
    from concourse import mybir
    from concourse.bass_utils import run_bass_kernel_spmd

    nc = bass.Bass()
    inp = nc.declare_dram_parameter("o_in", [N, 16], mybir.dt.float32, isOutput=False)
    outp = nc.declare_dram_parameter("o_out", [N, 16], mybir.dt.float32, isOutput=True)
    with (
        nc.Block() as block,
        nc.semaphore("dma_sem") as dma_sem,
    ):

        @block.sync
        def _(sync):
            sync.dma_start(out=outp[:], in_=inp[:]).then_inc(dma_sem, 16)
            sync.wait_ge(dma_sem, 16)

    in_maps = [{"o_in": np.ascontiguousarray(host_out[b])} for b in range(B)]
    if trace:
        res = run_bass_kernel_spmd(nc, in_maps, list(range(B)), trace=True)
        LAST_EXEC_NS = getattr(res, "exec_time_ns", None)
    else:
        res = run_bass_kernel_spmd(nc, in_maps, list(range(B)))
    return np.stack([np.asarray(res.results[b]["o_out"]) for b in range(B)], axis=0)


def kernel(**inputs):
    host_out = _host_compute(inputs)
    try:
        return _bass_stage(host_out)
    except Exception:
        return host_out



# revision 4
# speedup vs baseline: 2.9114x; 2.9114x over previous
"""Bass/Tile kernel for nn_PointCloudMPE: per-core pipeline for one batch
(coords [4096,3], features [4096,64]) -> out [4096,16].

Pipeline: pairwise -dist^2 via PE matmul (homogeneous 5-dim trick) ->
top-16 threshold via DVE max8/match_replace/max8 -> 0/1 neighbor mask ->
masked moment sums via PE matmuls (mask transposed on PE) -> 3x3 covariance
-> batched branch-free cyclic Jacobi eigensolver -> normals/invariants ->
MLPs on PE (transposed activations) -> assembled [4096,16] output.
"""

import numpy as np

import concourse.bass as bass
import concourse.bacc as bacc
import concourse.mybir as mybir
import concourse.tile as tile
from concourse.masks import make_identity

f32 = mybir.dt.float32
f32r = mybir.dt.float32r
AF = mybir.ActivationFunctionType
OP = mybir.AluOpType

N, K, NB = 4096, 16, 32  # points, knn, row blocks of 128
NEG_BIG = -1.0e30


def _sl(t, q):
    # [128, 32, C] tile -> [128, 32] strided view of component q
    return t[:, :, q : q + 1].squeeze(2)


def build_nc():
    nc = bacc.Bacc("TRN2", target_bir_lowering=False)

    coords_h = nc.declare_dram_parameter("coords", [N, 3], f32, isOutput=False)
    feats_h = nc.declare_dram_parameter("features", [N, 64], f32, isOutput=False)
    wnames = {
        "inv_W1": [6, 128], "inv_b1": [128], "inv_W2": [128, 128], "inv_b2": [128],
        "feat_W1": [64, 128], "feat_b1": [128], "feat_W2": [128, 128], "feat_b2": [128],
        "sh_W1": [256, 128], "sh_b1": [128], "sh_W2": [128, 128], "sh_b2": [128],
        "g0_W": [128, 1], "g0_b": [1],
    }
    wh = {k: nc.declare_dram_parameter(k, shp, f32, isOutput=False)
          for k, shp in wnames.items()}
    out_h = nc.declare_dram_parameter("out", [N, 16], f32, isOutput=True)

    with tile.TileContext(nc) as tc:
        _body(nc, tc, coords_h, feats_h, wh, out_h)
    nc.finalize()
    return nc


def _body(nc, tc, coords_h, feats_h, wh, out_h):
    import contextlib

    ctx = contextlib.ExitStack()
    with ctx:
        persist = ctx.enter_context(tc.tile_pool(name="persist", bufs=1))
        psum_small = ctx.enter_context(tc.tile_pool(name="ps_small", bufs=2, space="PSUM"))

        ident = persist.tile([128, 128], f32)
        make_identity(nc, ident[:])

        # ---- load coords in block layout + weights ----
        coords_nat = persist.tile([128, NB, 3], f32)
        nc.sync.dma_start(coords_nat[:],
                          coords_h[:, :].rearrange("(rb p) d -> p rb d", p=128))

        w = {}
        for k in ("inv_W1", "inv_W2", "feat_W1", "feat_W2", "sh_W2", "g0_W"):
            shp = [wh[k].shape[0], wh[k].shape[1]]
            w[k] = persist.tile(shp, f32, name=f"w_{k}")
            nc.sync.dma_start(w[k][:], wh[k][:, :])
        w["sh_W1a"] = persist.tile([128, 128], f32, name="w_sh_W1a")
        w["sh_W1b"] = persist.tile([128, 128], f32, name="w_sh_W1b")
        nc.sync.dma_start(w["sh_W1a"][:], wh["sh_W1"][0:128, :])
        nc.sync.dma_start(w["sh_W1b"][:], wh["sh_W1"][128:256, :])
        for k in ("inv_b1", "inv_b2", "feat_b1", "feat_b2", "sh_b1", "sh_b2"):
            w[k] = persist.tile([128, 1], f32, name=f"w_{k}")
            nc.sync.dma_start(w[k][:], wh[k][:].unsqueeze(1))
        g0b_one = persist.tile([1, 1], f32)
        nc.sync.dma_start(g0b_one[:], wh["g0_b"][:].unsqueeze(1))
        ones_row = persist.tile([1, 128], f32)
        nc.vector.memset(ones_row[:], 1.0)
        w["g0_b"] = persist.tile([128, 1], f32, name="w_g0_b_bc")
        g0ps = psum_small.tile([128, 1], f32, name="tp")
        nc.tensor.matmul(g0ps[:], ones_row[:], g0b_one[:], start=True, stop=True)
        nc.scalar.activation(w["g0_b"][:], g0ps[:], AF.Copy)

        # ---- phi [128, 32, 9]: x,y,z,xx,xy,xz,yy,yz,zz per point ----
        phi = persist.tile([128, NB, 9], f32)
        nc.vector.tensor_copy(phi[:, :, 0:3], coords_nat[:])
        pq = [(0, 0), (0, 1), (0, 2), (1, 1), (1, 2), (2, 2)]
        for qi, (a, b) in enumerate(pq):
            nc.vector.tensor_tensor(
                phi[:, :, 3 + qi : 4 + qi], coords_nat[:, :, a : a + 1],
                coords_nat[:, :, b : b + 1], OP.mult)

        # ---- L/R matrices for -dist^2 matmul, via block-layout staging ----
        # L col j: (x,y,z,sq,1); R col j: (2x,2y,2z,-1,-sq)
        sq_nat = persist.tile([128, NB], f32)
        nc.vector.tensor_add(sq_nat[:], _sl(phi, 3), _sl(phi, 6))
        nc.vector.tensor_add(sq_nat[:], sq_nat[:], _sl(phi, 8))
        Lstage = persist.tile([128, NB, 5], f32)
        Rstage = persist.tile([128, NB, 5], f32)
        nc.vector.tensor_copy(Lstage[:, :, 0:3], coords_nat[:])
        nc.vector.tensor_copy(_sl(Lstage, 3), sq_nat[:])
        nc.vector.memset(_sl(Lstage, 4), 1.0)
        nc.scalar.activation(Rstage[:, :, 0:3], coords_nat[:], AF.Copy, scale=2.0)
        nc.vector.memset(_sl(Rstage, 3), -1.0)
        nc.scalar.activation(_sl(Rstage, 4), sq_nat[:], AF.Copy, scale=-1.0)

        L_all = persist.tile([5, N], f32)
        R_all = persist.tile([5, N], f32)
        for rb in range(NB):
            tpl = psum_small.tile([5, 128], f32, name="tp")
            nc.tensor.transpose(tpl[:], Lstage[:, rb : rb + 1, :].squeeze(1), ident[:])
            nc.scalar.activation(L_all[:, rb * 128:(rb + 1) * 128], tpl[:], AF.Copy)
            tpr = psum_small.tile([5, 128], f32, name="tp")
            nc.tensor.transpose(tpr[:], Rstage[:, rb : rb + 1, :].squeeze(1), ident[:])
            nc.scalar.activation(R_all[:, rb * 128:(rb + 1) * 128], tpr[:], AF.Copy)

        # ---- featT [64, 4096] ----
        featT = persist.tile([64, N], f32)
        with tc.tile_pool(name="ftile", bufs=2) as fpool:
            for rb in range(NB):
                ft = fpool.tile([128, 64], f32)
                nc.sync.dma_start(ft[:], feats_h[rb * 128:(rb + 1) * 128, :])
                tp = psum_small.tile([64, 128], f32, name="tp")
                nc.tensor.transpose(tp[:], ft[:], ident[:])
                nc.scalar.activation(featT[:, rb * 128:(rb + 1) * 128], tp[:], AF.Copy)

        # ---- per-point accumulators (block layout [128, 32]) ----
        radii = persist.tile([128, NB], f32)
        Sall = persist.tile([128, 9, NB], f32)

        # ---- main selection + moment loop over row blocks ----
        with (
            tc.tile_pool(name="ndpool", bufs=2) as ndpool,
            tc.tile_pool(name="scrpool", bufs=1) as scrpool,
            tc.tile_pool(name="mpool", bufs=1) as mpool,
            tc.tile_pool(name="mtpool", bufs=1) as mtpool,
            tc.tile_pool(name="selpool", bufs=2) as selpool,
            tc.tile_pool(name="ps_nd", bufs=2, space="PSUM") as ps_nd,
            tc.tile_pool(name="ps_mt", bufs=2, space="PSUM") as ps_mt,
            tc.tile_pool(name="ps_s", bufs=1, space="PSUM") as ps_s,
        ):
            for rb in range(NB):
                nd = ndpool.tile([128, N], f32)
                lhs = L_all[:, rb * 128:(rb + 1) * 128]
                for ct in range(8):
                    ps = ps_nd.tile([128, 512], f32)
                    nc.tensor.matmul(ps[:], lhs, R_all[:, ct * 512:(ct + 1) * 512],
                                     start=True, stop=True)
                    nc.scalar.activation(nd[:, ct * 512:(ct + 1) * 512], ps[:], AF.Copy)
                # self-exclusion: diagonal block columns rb*128..rb*128+127
                nc.gpsimd.affine_select(
                    out=nd[:, rb * 128:(rb + 1) * 128],
                    in_=nd[:, rb * 128:(rb + 1) * 128],
                    compare_op=OP.not_equal, fill=NEG_BIG,
                    base=0, pattern=[[1, 128]], channel_multiplier=-1)

                mx1 = selpool.tile([128, 8], f32)
                mx2 = selpool.tile([128, 8], f32)
                scr = scrpool.tile([128, N], f32)
                nc.vector.max(out=mx1[:], in_=nd[:])
                nc.vector.match_replace(out=scr[:], in_to_replace=mx1[:],
                                        in_values=nd[:], imm_value=NEG_BIG)
                nc.vector.max(out=mx2[:], in_=scr[:])

                mask = mpool.tile([128, N], f32)
                nc.gpsimd.tensor_scalar(mask[:], nd[:], mx2[:, 7:8], None, op0=OP.is_ge)

                # radius: sum over selected of sqrt(dist), NaN-proof:
                # scr = min(nd,0)*mask (<=0), then sqrt(-scr) with accumulate
                nc.vector.scalar_tensor_tensor(
                    out=scr[:], in0=nd[:], scalar=0.0, in1=mask[:],
                    op0=OP.min, op1=OP.mult)
                nc.scalar.activation(scr[:], scr[:], AF.Sqrt, scale=-1.0,
                                     accum_out=radii[:, rb : rb + 1])

                # transpose mask strip -> MT [j, i] for this row block
                mt = mtpool.tile([128, N], f32)
                for g in range(8):
                    tp = ps_mt.tile([128, 512], f32)
                    for kk in range(4):
                        c0 = (4 * g + kk) * 128
                        nc.tensor.transpose(tp[:, kk * 128:(kk + 1) * 128],
                                            mask[:, c0:c0 + 128], ident[:])
                    nc.scalar.activation(mt[:, g * 512:(g + 1) * 512], tp[:], AF.Copy)

                sps = ps_s.tile([128, 9], f32)
                for jb in range(NB):
                    nc.tensor.matmul(sps[:], mt[:, jb * 128:(jb + 1) * 128],
                                     phi[:, jb : jb + 1, :].squeeze(1), start=(jb == 0), stop=(jb == NB - 1))
                nc.scalar.activation(Sall[:, :, rb : rb + 1].squeeze(2), sps[:], AF.Copy)

        # ================= covariance + Jacobi + invariants =================
        jp = ctx.enter_context(tc.tile_pool(name="jacobi", bufs=1))

        _jtc = [0]

        def jt():
            _jtc[0] += 1
            return jp.tile([128, NB], f32, name=f"jt{_jtc[0]}")

        cx, cy, cz = (_sl(coords_nat, q) for q in range(3))
        inv_k = 1.0 / K

        # neighbor means
        m1 = [jp.tile([128, NB], f32, name=f"m1_{q}") for q in range(3)]
        for q in range(3):
            nc.scalar.activation(m1[q][:], Sall[:, q : q + 1, :].squeeze(1), AF.Copy, scale=inv_k)

        # cov components: S2ab/K - ca*m1b - cb*m1a + ca*cb
        cvs = [cx, cy, cz]
        A = {}
        for qi, (a, b) in enumerate(pq):
            t0 = jt()
            nc.scalar.activation(t0[:], Sall[:, 3 + qi : 4 + qi, :].squeeze(1), AF.Copy, scale=inv_k)
            t1 = jt()
            nc.vector.tensor_tensor(t1[:], cvs[a], m1[b][:], OP.mult)
            nc.vector.tensor_sub(t0[:], t0[:], t1[:])
            nc.vector.tensor_tensor(t1[:], cvs[b], m1[a][:], OP.mult)
            nc.vector.tensor_sub(t0[:], t0[:], t1[:])
            nc.vector.tensor_tensor(t1[:], cvs[a], cvs[b], OP.mult)
            nc.vector.tensor_add(t0[:], t0[:], t1[:])
            A[(a, b)] = t0

        # V = I (9 tiles)
        V = {}
        for i in range(3):
            for j in range(3):
                V[(i, j)] = jp.tile([128, NB], f32, name=f"V{i}{j}")
                nc.vector.memset(V[(i, j)][:], 1.0 if i == j else 0.0)

        def Ag(i, j):
            return A[(i, j)] if i <= j else A[(j, i)]

        tmp1, tmp2, tmp3, tmp4 = jt(), jt(), jt(), jt()
        SWEEPS = 4
        for sw in range(SWEEPS):
            for (p, q) in ((0, 1), (0, 2), (1, 2)):
                app, aqq, apq = Ag(p, p), Ag(q, q), Ag(p, q)
                # theta = (aqq - app) / (2*apq), guarded; t = sgn/( |th| + sqrt(th^2+1) )
                sA = tmp1
                nc.vector.tensor_scalar(sA[:], apq[:], 0.0, None, op0=OP.is_ge)
                nc.vector.tensor_scalar(sA[:], sA[:], 2.0, -1.0, op0=OP.mult, op1=OP.add)
                den = tmp2
                nc.vector.tensor_tensor(den[:], apq[:], sA[:], OP.mult)  # |apq|
                nc.vector.tensor_scalar(den[:], den[:], 1e-30, 2.0, op0=OP.max, op1=OP.mult)
                nc.vector.reciprocal(den[:], den[:])                     # 1/(2|apq|)
                th = tmp3
                nc.vector.tensor_sub(th[:], app[:], aqq[:])
                nc.vector.tensor_tensor(th[:], th[:], den[:], OP.mult)
                nc.vector.tensor_tensor(th[:], th[:], sA[:], OP.mult)    # theta (signed)
                nc.vector.tensor_scalar(th[:], th[:], 1e17, -1e17, op0=OP.min, op1=OP.max)
                sT = tmp1
                nc.vector.tensor_scalar(sT[:], th[:], 0.0, None, op0=OP.is_ge)
                nc.vector.tensor_scalar(sT[:], sT[:], 2.0, -1.0, op0=OP.mult, op1=OP.add)
                tha = tmp2
                nc.vector.tensor_tensor(tha[:], th[:], sT[:], OP.mult)   # |theta|
                r_ = tmp4
                nc.vector.tensor_tensor(r_[:], th[:], th[:], OP.mult)
                nc.scalar.activation(r_[:], r_[:], AF.Sqrt, bias=1.0)    # sqrt(th^2+1)
                nc.vector.tensor_add(r_[:], r_[:], tha[:])
                nc.vector.reciprocal(r_[:], r_[:])
                tt = tmp2
                nc.vector.tensor_tensor(tt[:], r_[:], sT[:], OP.mult)    # tan(phi)
                cc_ = tmp3
                nc.vector.tensor_tensor(cc_[:], tt[:], tt[:], OP.mult)
                nc.scalar.activation(cc_[:], cc_[:], AF.Sqrt, bias=1.0)
                nc.vector.reciprocal(cc_[:], cc_[:])                     # cos(phi)
                ss_ = tmp4
                nc.vector.tensor_tensor(ss_[:], tt[:], cc_[:], OP.mult)  # sin(phi)
                cs, sn = cc_, ss_

                # update A
                c2, s2, csn = jt(), jt(), jt()
                nc.vector.tensor_tensor(c2[:], cs[:], cs[:], OP.mult)
                nc.vector.tensor_tensor(s2[:], sn[:], sn[:], OP.mult)
                nc.vector.tensor_tensor(csn[:], cs[:], sn[:], OP.mult)
                u1, u2, u3, u4 = jt(), jt(), jt(), jt()
                nc.vector.tensor_tensor(u1[:], c2[:], app[:], OP.mult)
                nc.vector.tensor_tensor(u2[:], csn[:], apq[:], OP.mult)
                nc.vector.tensor_tensor(u3[:], s2[:], aqq[:], OP.mult)
                nc.vector.tensor_tensor(u4[:], s2[:], app[:], OP.mult)
                nc.gpsimd.tensor_tensor(app[:], c2[:], aqq[:], OP.mult)  # app <- c2*aqq (tmp)
                # app' = u1 + 2*u2 + u3 ; aqq' = u4 - 2*u2 + c2*aqq
                nc.vector.scalar_tensor_tensor(u1[:], u2[:], 2.0, u1[:], op0=OP.mult, op1=OP.add)
                nc.vector.tensor_add(u1[:], u1[:], u3[:])
                nc.vector.scalar_tensor_tensor(u4[:], u2[:], -2.0, u4[:], op0=OP.mult, op1=OP.add)
                nc.vector.tensor_add(aqq[:], u4[:], app[:])
                nc.vector.tensor_copy(app[:], u1[:])
                nc.vector.memset(apq[:], 0.0)
                rr = 3 - p - q
                apr, aqr = Ag(p, rr), Ag(q, rr)
                nc.vector.tensor_tensor(u1[:], cs[:], apr[:], OP.mult)
                nc.vector.tensor_tensor(u2[:], sn[:], aqr[:], OP.mult)
                nc.vector.tensor_tensor(u3[:], sn[:], apr[:], OP.mult)
                nc.vector.tensor_tensor(u4[:], cs[:], aqr[:], OP.mult)
                nc.vector.tensor_add(apr[:], u1[:], u2[:])
                nc.vector.tensor_sub(aqr[:], u4[:], u3[:])
                # update V columns p,q (rows 0..2) on gpsimd
                for i in range(3):
                    vip, viq = V[(i, p)], V[(i, q)]
                    nc.gpsimd.tensor_tensor(u1[:], cs[:], vip[:], OP.mult)
                    nc.gpsimd.tensor_tensor(u2[:], sn[:], viq[:], OP.mult)
                    nc.gpsimd.tensor_tensor(u3[:], sn[:], vip[:], OP.mult)
                    nc.gpsimd.tensor_tensor(u4[:], cs[:], viq[:], OP.mult)
                    nc.gpsimd.tensor_add(vip[:], u1[:], u2[:])
                    nc.gpsimd.tensor_sub(viq[:], u4[:], u3[:])

        e0, e1, e2 = A[(0, 0)], A[(1, 1)], A[(2, 2)]
        smin, smid, smax = jt(), jt(), jt()
        nc.vector.tensor_tensor(smin[:], e0[:], e1[:], OP.min)
        nc.vector.tensor_tensor(smin[:], smin[:], e2[:], OP.min)
        nc.vector.tensor_tensor(smax[:], e0[:], e1[:], OP.max)
        nc.vector.tensor_tensor(smax[:], smax[:], e2[:], OP.max)
        nc.vector.tensor_add(smid[:], e0[:], e1[:])
        nc.vector.tensor_add(smid[:], smid[:], e2[:])
        esum = jt()
        nc.vector.tensor_copy(esum[:], smid[:])  # e0+e1+e2
        nc.vector.tensor_sub(smid[:], smid[:], smin[:])
        nc.vector.tensor_sub(smid[:], smid[:], smax[:])

        # select eigenvector of smallest eigenvalue: exclusive masks
        m0, m1m, m2m = jt(), jt(), jt()
        nc.vector.tensor_tensor(tmp1[:], e0[:], e1[:], OP.is_le)
        nc.vector.tensor_tensor(tmp2[:], e0[:], e2[:], OP.is_le)
        nc.vector.tensor_tensor(m0[:], tmp1[:], tmp2[:], OP.mult)
        nc.vector.tensor_tensor(tmp1[:], e1[:], e0[:], OP.is_lt)
        nc.vector.tensor_tensor(tmp2[:], e1[:], e2[:], OP.is_le)
        nc.vector.tensor_tensor(m1m[:], tmp1[:], tmp2[:], OP.mult)
        nc.vector.tensor_add(m2m[:], m0[:], m1m[:])
        nc.vector.tensor_scalar(m2m[:], m2m[:], -1.0, 1.0, op0=OP.mult, op1=OP.add)

        nrm = [jp.tile([128, NB], f32, name=f"nrm{q}") for q in range(3)]  # normals nx,ny,nz
        for i in range(3):
            nc.vector.tensor_tensor(tmp1[:], m0[:], V[(i, 0)][:], OP.mult)
            nc.vector.tensor_tensor(tmp2[:], m1m[:], V[(i, 1)][:], OP.mult)
            nc.vector.tensor_add(tmp1[:], tmp1[:], tmp2[:])
            nc.vector.tensor_tensor(tmp2[:], m2m[:], V[(i, 2)][:], OP.mult)
            nc.vector.tensor_add(nrm[i][:], tmp1[:], tmp2[:])

        # center + outward + orientation
        ones128 = jp.tile([128, 1], f32)
        nc.vector.memset(ones128[:], 1.0)
        partials = jp.tile([128, 3], f32)
        for q, cv in enumerate(cvs):
            nc.vector.tensor_reduce(partials[:, q : q + 1], cv, axis=mybir.AxisListType.X,
                                    op=OP.add)
        cps = psum_small.tile([1, 3], f32, name="tp")
        nc.tensor.matmul(cps[:], ones128[:], partials[:], start=True, stop=True)
        center1 = jp.tile([1, 3], f32)
        nc.scalar.activation(center1[:], cps[:], AF.Copy, scale=1.0 / N)
        center = jp.tile([128, 3], f32)
        cbps = psum_small.tile([128, 3], f32, name="tp")
        nc.tensor.matmul(cbps[:], ones_row[:], center1[:], start=True, stop=True)
        nc.scalar.activation(center[:], cbps[:], AF.Copy)

        ox, oy, oz = jt(), jt(), jt()
        ovs = [ox, oy, oz]
        for q in range(3):
            nc.vector.tensor_scalar(ovs[q][:], cvs[q], center[:, q : q + 1], None,
                                    op0=OP.subtract)
        dt_ = jt()
        nc.vector.tensor_tensor(dt_[:], nrm[0][:], ox[:], OP.mult)
        nc.vector.tensor_tensor(tmp1[:], nrm[1][:], oy[:], OP.mult)
        nc.vector.tensor_add(dt_[:], dt_[:], tmp1[:])
        nc.vector.tensor_tensor(tmp1[:], nrm[2][:], oz[:], OP.mult)
        nc.vector.tensor_add(dt_[:], dt_[:], tmp1[:])
        nc.vector.tensor_scalar(dt_[:], dt_[:], 0.0, None, op0=OP.is_ge)
        nc.vector.tensor_scalar(dt_[:], dt_[:], 2.0, -1.0, op0=OP.mult, op1=OP.add)
        for i in range(3):
            nc.vector.tensor_tensor(nrm[i][:], nrm[i][:], dt_[:], OP.mult)
        # normalize
        nc.vector.tensor_tensor(tmp1[:], nrm[0][:], nrm[0][:], OP.mult)
        nc.vector.tensor_tensor(tmp2[:], nrm[1][:], nrm[1][:], OP.mult)
        nc.vector.tensor_add(tmp1[:], tmp1[:], tmp2[:])
        nc.vector.tensor_tensor(tmp2[:], nrm[2][:], nrm[2][:], OP.mult)
        nc.vector.tensor_add(tmp1[:], tmp1[:], tmp2[:])
        nc.scalar.activation(tmp1[:], tmp1[:], AF.Sqrt)
        nc.vector.tensor_scalar(tmp1[:], tmp1[:], 1e-6, None, op0=OP.max)
        nc.vector.reciprocal(tmp1[:], tmp1[:])
        for i in range(3):
            nc.vector.tensor_tensor(nrm[i][:], nrm[i][:], tmp1[:], OP.mult)

        # centered radius + dominance + plane offset
        cr = jt()
        nc.vector.tensor_tensor(cr[:], ox[:], ox[:], OP.mult)
        nc.vector.tensor_tensor(tmp1[:], oy[:], oy[:], OP.mult)
        nc.vector.tensor_add(cr[:], cr[:], tmp1[:])
        nc.vector.tensor_tensor(tmp1[:], oz[:], oz[:], OP.mult)
        nc.vector.tensor_add(cr[:], cr[:], tmp1[:])
        nc.scalar.activation(cr[:], cr[:], AF.Sqrt)
        dom = jt()
        nc.vector.tensor_scalar(tmp1[:], esum[:], 1e-6, None, op0=OP.max)
        nc.vector.reciprocal(tmp1[:], tmp1[:])
        nc.vector.tensor_tensor(dom[:], smax[:], tmp1[:], OP.mult)
        pd = jt()  # + sum(c*n); negated at assembly
        nc.vector.tensor_tensor(pd[:], cx, nrm[0][:], OP.mult)
        nc.vector.tensor_tensor(tmp1[:], cy, nrm[1][:], OP.mult)
        nc.vector.tensor_add(pd[:], pd[:], tmp1[:])
        nc.vector.tensor_tensor(tmp1[:], cz, nrm[2][:], OP.mult)
        nc.vector.tensor_add(pd[:], pd[:], tmp1[:])

        # ================= MLPs (transposed activations) =================
        inv_stage = jp.tile([128, NB, 6], f32)
        nc.vector.tensor_copy(_sl(inv_stage, 0), smin[:])
        nc.vector.tensor_copy(_sl(inv_stage, 1), smid[:])
        nc.vector.tensor_copy(_sl(inv_stage, 2), smax[:])
        nc.scalar.activation(_sl(inv_stage, 3), radii[:], AF.Copy, scale=1.0 / K)
        nc.vector.tensor_copy(_sl(inv_stage, 4), cr[:])
        nc.vector.tensor_copy(_sl(inv_stage, 5), dom[:])

        invT = jp.tile([6, N], f32)
        for rb in range(NB):
            tp = psum_small.tile([6, 128], f32, name="tp")
            nc.tensor.transpose(tp[:], inv_stage[:, rb : rb + 1, :].squeeze(1), ident[:])
            nc.scalar.activation(invT[:, rb * 128:(rb + 1) * 128], tp[:], AF.Copy)

        hidden = jp.tile([128, N], f32)
        outstage = jp.tile([128, NB, 16], f32)
        nc.vector.memset(outstage[:], 0.0)

        with (
            tc.tile_pool(name="mlp_sb", bufs=3) as mlp_sb,
            tc.tile_pool(name="ps_mlp", bufs=4, space="PSUM") as ps_mlp,
        ):
            for ch in range(8):
                cslice = slice(ch * 512, (ch + 1) * 512)
                # inv branch
                p1 = ps_mlp.tile([128, 512], f32, name="pmlp")
                nc.tensor.matmul(p1[:], w["inv_W1"][:],
                                 invT[:, cslice], start=True, stop=True)
                h1 = mlp_sb.tile([128, 512], f32)
                nc.scalar.activation(h1[:], p1[:], AF.Gelu, bias=w["inv_b1"][:])
                p2 = ps_mlp.tile([128, 512], f32, name="pmlp")
                nc.tensor.matmul(p2[:], w["inv_W2"][:],
                                 h1[:], start=True, stop=True)
                ih = mlp_sb.tile([128, 512], f32)
                nc.scalar.activation(ih[:], p2[:], AF.Identity, bias=w["inv_b2"][:])
                # feat branch
                p3 = ps_mlp.tile([128, 512], f32, name="pmlp")
                nc.tensor.matmul(p3[:], w["feat_W1"][:],
                                 featT[:, cslice], start=True, stop=True)
                h2 = mlp_sb.tile([128, 512], f32)
                nc.scalar.activation(h2[:], p3[:], AF.Gelu, bias=w["feat_b1"][:])
                p4 = ps_mlp.tile([128, 512], f32, name="pmlp")
                nc.tensor.matmul(p4[:], w["feat_W2"][:],
                                 h2[:], start=True, stop=True)
                fh = mlp_sb.tile([128, 512], f32)
                nc.scalar.activation(fh[:], p4[:], AF.Identity, bias=w["feat_b2"][:])
                # shared
                p5 = ps_mlp.tile([128, 512], f32, name="pmlp")
                nc.tensor.matmul(p5[:], w["sh_W1a"][:], ih[:],
                                 start=True, stop=False)
                nc.tensor.matmul(p5[:], w["sh_W1b"][:], fh[:],
                                 start=False, stop=True)
                hs = mlp_sb.tile([128, 512], f32)
                nc.scalar.activation(hs[:], p5[:], AF.Gelu, bias=w["sh_b1"][:])
                p6 = ps_mlp.tile([128, 512], f32, name="pmlp")
                nc.tensor.matmul(p6[:], w["sh_W2"][:], hs[:],
                                 start=True, stop=True)
                nc.scalar.activation(hidden[:, cslice], p6[:], AF.Identity,
                                     bias=w["sh_b2"][:])

            # scalar head: per row block, lhsT = hidden strip, rhs = g0_W
            for rb in range(NB):
                sp = ps_mlp.tile([128, 1], f32, name="psp", bufs=2)
                nc.tensor.matmul(sp[:], hidden[:, rb * 128:(rb + 1) * 128], w["g0_W"][:],
                                 start=True, stop=True)
                nc.scalar.activation(outstage[:, rb : rb + 1, 0:1].squeeze(1), sp[:], AF.Identity,
                                     bias=w["g0_b"][:])

        # ================= assembly + output DMA =================
        nc.vector.tensor_copy(_sl(outstage, 1), nrm[0][:])
        nc.vector.tensor_copy(_sl(outstage, 2), nrm[1][:])
        nc.scalar.activation(_sl(outstage, 3), nrm[2][:], AF.Copy, scale=-1.0)
        nc.scalar.activation(_sl(outstage, 4), pd[:], AF.Copy, scale=-1.0)
        nc.vector.tensor_copy(_sl(outstage, 5), nrm[0][:])
        nc.vector.tensor_copy(_sl(outstage, 6), nrm[1][:])
        nc.vector.tensor_copy(_sl(outstage, 7), nrm[2][:])
        nc.vector.tensor_copy(_sl(outstage, 11), cx)
        nc.vector.tensor_copy(_sl(outstage, 12), cy)
        nc.vector.tensor_copy(_sl(outstage, 13), cz)
        nc.vector.memset(_sl(outstage, 14), 1.0)

        for rb in range(NB):
            nc.sync.dma_start(out_h[rb * 128:(rb + 1) * 128, :], outstage[:, rb : rb + 1, :].squeeze(1))


# ---------------------------------------------------------------------------
# SPMD driver: batch b -> core b (persistent PJRT executable)
# ---------------------------------------------------------------------------
_WKEYS = ("inv_W1", "inv_b1", "inv_W2", "inv_b2", "feat_W1", "feat_b1",
          "feat_W2", "feat_b2", "sh_W1", "sh_b1", "sh_W2", "sh_b2",
          "g0_W", "g0_b")
_B = 8
_CACHE = {}


def _get_runner():
    if "runner" in _CACHE:
        return _CACHE["runner"]
    import jax
    from jax.sharding import Mesh, PartitionSpec
    from jax.experimental.shard_map import shard_map
    from concourse import bass2jax, mybir as _mb
    from concourse.bass2jax import _bass_exec_p, install_neuronx_cc_hook

    install_neuronx_cc_hook()
    nc = build_nc()
    partition_name = nc.partition_id_tensor.name if nc.partition_id_tensor else None
    in_names, out_names, out_avals = [], [], []
    for alloc in nc.m.functions[0].allocations:
        if not isinstance(alloc, _mb.MemoryLocationSet):
            continue
        name = alloc.memorylocations[0].name
        if alloc.kind == "ExternalInput":
            if name != partition_name:
                in_names.append(name)
        elif alloc.kind == "ExternalOutput":
            shape = tuple(alloc.tensor_shape)
            out_avals.append(jax.core.ShapedArray(shape, _mb.dt.np(alloc.dtype)))
            out_names.append(name)
    n_params = len(in_names)
    n_outs = len(out_avals)
    all_in_names = list(in_names) + list(out_names)
    if partition_name is not None:
        all_in_names.append(partition_name)

    def _body(*args):
        operands = list(args)
        if partition_name is not None:
            operands.append(bass2jax.partition_id_tensor())
        return tuple(_bass_exec_p.bind(
            *operands,
            out_avals=tuple(out_avals),
            in_names=tuple(all_in_names),
            out_names=tuple(out_names),
            lowering_input_output_aliases=(),
            sim_require_finite=True,
            sim_require_nnan=True,
            nc=nc,
        ))

    devices = jax.devices()[:_B]
    mesh = Mesh(np.asarray(devices), ("core",))
    donate = tuple(range(n_params, n_params + n_outs))
    sharded = jax.jit(
        shard_map(_body, mesh=mesh,
                  in_specs=(PartitionSpec("core"),) * (n_params + n_outs),
                  out_specs=(PartitionSpec("core"),) * n_outs,
                  check_rep=False),
        donate_argnums=donate, keep_unused=True)
    _CACHE["runner"] = (sharded, in_names, out_names, out_avals)
    return _CACHE["runner"]


def _concat_inputs(inputs, in_names):
    per_core = []
    for b in range(_B):
        m = {
            "coords": np.ascontiguousarray(inputs["coords"][b], dtype=np.float32),
            "features": np.ascontiguousarray(inputs["features"][b], dtype=np.float32),
        }
        for k in _WKEYS:
            m[k] = np.ascontiguousarray(np.asarray(inputs[k], dtype=np.float32))
        per_core.append([m[n] for n in in_names])
    return [np.concatenate([per_core[c][i] for c in range(_B)], axis=0)
            for i in range(len(in_names))]


def _zeros_for(out_avals):
    return [np.zeros((_B * a.shape[0],) + tuple(a.shape[1:]), a.dtype)
            for a in out_avals]


def run(inputs):
    sharded, in_names, out_names, out_avals = _get_runner()
    concat_in = _concat_inputs(inputs, in_names)
    outs = sharded(*concat_in, *_zeros_for(out_avals))
    i = out_names.index("out")
    full = np.asarray(outs[i]).reshape(_B, *out_avals[i].shape)
    return full.astype(np.float32)


def time_kernel(inputs, iters=10):
    """Median wall time per steady-state invocation (ns)."""
    import time as _time
    import jax
    sharded, in_names, out_names, out_avals = _get_runner()
    concat_in = [jax.device_put(a) for a in _concat_inputs(inputs, in_names)]
    ts = []
    for _ in range(iters + 2):
        z = _zeros_for(out_avals)
        t0 = _time.perf_counter()
        outs = sharded(*concat_in, *z)
        jax.block_until_ready(outs)
        ts.append((_time.perf_counter() - t0) * 1e9)
    ts = sorted(ts[2:])
    return ts[len(ts) // 2]


def kernel(**inputs):
    return run(inputs)


# revision 5
# speedup vs baseline: 12.4132x; 4.2637x over previous
"""Bass/Tile kernel for nn_PointCloudMPE: per-core pipeline for one batch
(coords [4096,3], features [4096,64]) -> out [4096,16].

Pipeline: pairwise -dist^2 via PE matmul (homogeneous 5-dim trick) ->
top-16 threshold via DVE max8/match_replace/max8 -> 0/1 neighbor mask ->
masked moment sums via PE matmuls (mask transposed on PE) -> 3x3 covariance
-> batched branch-free cyclic Jacobi eigensolver -> normals/invariants ->
MLPs on PE (transposed activations) -> assembled [4096,16] output.
"""

import numpy as np

import concourse.bass as bass
import concourse.bacc as bacc
import concourse.mybir as mybir
import concourse.tile as tile
from concourse.masks import make_identity

f32 = mybir.dt.float32
f32r = mybir.dt.float32r
AF = mybir.ActivationFunctionType
OP = mybir.AluOpType

N, K, NB = 4096, 16, 32  # points, knn, row blocks of 128
NEG_BIG = -1.0e30


def _sl(t, q):
    # [128, 32, C] tile -> [128, 32] strided view of component q
    return t[:, :, q : q + 1].squeeze(2)


def build_nc():
    nc = bacc.Bacc("TRN2", target_bir_lowering=False)

    coords_h = nc.declare_dram_parameter("coords", [N, 3], f32, isOutput=False)
    feats_h = nc.declare_dram_parameter("features", [N, 64], f32, isOutput=False)
    wnames = {
        "inv_W1": [6, 128], "inv_b1": [128], "inv_W2": [128, 128], "inv_b2": [128],
        "feat_W1": [64, 128], "feat_b1": [128], "feat_W2": [128, 128], "feat_b2": [128],
        "sh_W1": [256, 128], "sh_b1": [128], "sh_W2": [128, 128], "sh_b2": [128],
        "g0_W": [128, 1], "g0_b": [1],
    }
    wh = {k: nc.declare_dram_parameter(k, shp, f32, isOutput=False)
          for k, shp in wnames.items()}
    out_h = nc.declare_dram_parameter("out", [N, 16], f32, isOutput=True)

    with tile.TileContext(nc) as tc:
        _body(nc, tc, coords_h, feats_h, wh, out_h)
    nc.finalize()
    return nc


def _body(nc, tc, coords_h, feats_h, wh, out_h):
    import contextlib

    ctx = contextlib.ExitStack()
    with ctx:
        persist = ctx.enter_context(tc.tile_pool(name="persist", bufs=1))
        psum_small = ctx.enter_context(tc.tile_pool(name="ps_small", bufs=2, space="PSUM"))

        ident = persist.tile([128, 128], f32)
        make_identity(nc, ident[:])

        # ---- load coords in block layout + weights ----
        coords_nat = persist.tile([128, NB, 3], f32)
        nc.sync.dma_start(coords_nat[:],
                          coords_h[:, :].rearrange("(rb p) d -> p rb d", p=128))

        w = {}
        for k in ("inv_W1", "inv_W2", "feat_W1", "feat_W2", "sh_W2", "g0_W"):
            shp = [wh[k].shape[0], wh[k].shape[1]]
            w[k] = persist.tile(shp, f32, name=f"w_{k}")
            nc.sync.dma_start(w[k][:], wh[k][:, :])
        w["sh_W1a"] = persist.tile([128, 128], f32, name="w_sh_W1a")
        w["sh_W1b"] = persist.tile([128, 128], f32, name="w_sh_W1b")
        nc.sync.dma_start(w["sh_W1a"][:], wh["sh_W1"][0:128, :])
        nc.sync.dma_start(w["sh_W1b"][:], wh["sh_W1"][128:256, :])
        for k in ("inv_b1", "inv_b2", "feat_b1", "feat_b2", "sh_b1", "sh_b2"):
            w[k] = persist.tile([128, 1], f32, name=f"w_{k}")
            nc.sync.dma_start(w[k][:], wh[k][:].unsqueeze(1))
        g0b_one = persist.tile([1, 1], f32)
        nc.sync.dma_start(g0b_one[:], wh["g0_b"][:].unsqueeze(1))
        ones_row = persist.tile([1, 128], f32)
        nc.vector.memset(ones_row[:], 1.0)
        w["g0_b"] = persist.tile([128, 1], f32, name="w_g0_b_bc")
        g0ps = psum_small.tile([128, 1], f32, name="tp")
        nc.tensor.matmul(g0ps[:], ones_row[:], g0b_one[:], start=True, stop=True)
        nc.scalar.activation(w["g0_b"][:], g0ps[:], AF.Copy)

        # ---- phi [128, 32, 9]: x,y,z,xx,xy,xz,yy,yz,zz per point ----
        phi = persist.tile([128, NB, 9], f32)
        nc.vector.tensor_copy(phi[:, :, 0:3], coords_nat[:])
        pq = [(0, 0), (0, 1), (0, 2), (1, 1), (1, 2), (2, 2)]
        for qi, (a, b) in enumerate(pq):
            nc.vector.tensor_tensor(
                phi[:, :, 3 + qi : 4 + qi], coords_nat[:, :, a : a + 1],
                coords_nat[:, :, b : b + 1], OP.mult)

        # ---- L/R matrices for -dist^2 matmul, via block-layout staging ----
        # L col j: (x,y,z,sq,1); R col j: (2x,2y,2z,-1,-sq)
        sq_nat = persist.tile([128, NB], f32)
        nc.vector.tensor_add(sq_nat[:], _sl(phi, 3), _sl(phi, 6))
        nc.vector.tensor_add(sq_nat[:], sq_nat[:], _sl(phi, 8))
        Lstage = persist.tile([128, NB, 5], f32)
        Rstage = persist.tile([128, NB, 5], f32)
        nc.vector.tensor_copy(Lstage[:, :, 0:3], coords_nat[:])
        nc.vector.tensor_copy(_sl(Lstage, 3), sq_nat[:])
        nc.vector.memset(_sl(Lstage, 4), 1.0)
        nc.scalar.activation(Rstage[:, :, 0:3], coords_nat[:], AF.Copy, scale=2.0)
        nc.vector.memset(_sl(Rstage, 3), -1.0)
        nc.scalar.activation(_sl(Rstage, 4), sq_nat[:], AF.Copy, scale=-1.0)

        L_all = persist.tile([5, N], f32)
        R_all = persist.tile([5, N], f32)
        for rb in range(NB):
            tpl = psum_small.tile([5, 128], f32, name="tp")
            nc.tensor.transpose(tpl[:], Lstage[:, rb : rb + 1, :].squeeze(1), ident[:])
            nc.scalar.activation(L_all[:, rb * 128:(rb + 1) * 128], tpl[:], AF.Copy)
            tpr = psum_small.tile([5, 128], f32, name="tp")
            nc.tensor.transpose(tpr[:], Rstage[:, rb : rb + 1, :].squeeze(1), ident[:])
            nc.scalar.activation(R_all[:, rb * 128:(rb + 1) * 128], tpr[:], AF.Copy)

        # ---- featT [64, 4096] ----
        featT = persist.tile([64, N], f32)
        with tc.tile_pool(name="ftile", bufs=2) as fpool:
            for rb in range(NB):
                ft = fpool.tile([128, 64], f32)
                nc.sync.dma_start(ft[:], feats_h[rb * 128:(rb + 1) * 128, :])
                tp = psum_small.tile([64, 128], f32, name="tp")
                nc.tensor.transpose(tp[:], ft[:], ident[:])
                nc.scalar.activation(featT[:, rb * 128:(rb + 1) * 128], tp[:], AF.Copy)

        # ---- per-point accumulators (block layout [128, 32]) ----
        radii = persist.tile([128, NB], f32)
        Sall = persist.tile([128, 9, NB], f32)

        # ---- main selection + moment loop over row blocks ----
        with (
            tc.tile_pool(name="ndpool", bufs=2) as ndpool,
            tc.tile_pool(name="scrpool", bufs=1) as scrpool,
            tc.tile_pool(name="mpool", bufs=1) as mpool,
            tc.tile_pool(name="mtpool", bufs=1) as mtpool,
            tc.tile_pool(name="selpool", bufs=2) as selpool,
            tc.tile_pool(name="ps_nd", bufs=2, space="PSUM") as ps_nd,
            tc.tile_pool(name="ps_mt", bufs=2, space="PSUM") as ps_mt,
            tc.tile_pool(name="ps_s", bufs=1, space="PSUM") as ps_s,
        ):
            for rb in range(NB):
                nd = ndpool.tile([128, N], f32)
                lhs = L_all[:, rb * 128:(rb + 1) * 128]
                for ct in range(8):
                    ps = ps_nd.tile([128, 512], f32)
                    nc.tensor.matmul(ps[:], lhs, R_all[:, ct * 512:(ct + 1) * 512],
                                     start=True, stop=True)
                    nc.scalar.activation(nd[:, ct * 512:(ct + 1) * 512], ps[:], AF.Copy)
                # self-exclusion: diagonal block columns rb*128..rb*128+127
                nc.gpsimd.affine_select(
                    out=nd[:, rb * 128:(rb + 1) * 128],
                    in_=nd[:, rb * 128:(rb + 1) * 128],
                    compare_op=OP.not_equal, fill=NEG_BIG,
                    base=0, pattern=[[1, 128]], channel_multiplier=-1)

                mx1 = selpool.tile([128, 8], f32)
                mx2 = selpool.tile([128, 8], f32)
                scr = scrpool.tile([128, N], f32)
                nc.vector.max(out=mx1[:], in_=nd[:])
                nc.vector.match_replace(out=scr[:], in_to_replace=mx1[:],
                                        in_values=nd[:], imm_value=NEG_BIG)
                nc.vector.max(out=mx2[:], in_=scr[:])

                mask = mpool.tile([128, N], f32)
                nc.gpsimd.tensor_scalar(mask[:], nd[:], mx2[:, 7:8], None, op0=OP.is_ge)

                # radius: sum over selected of sqrt(dist), NaN-proof:
                # scr = min(nd,0)*mask (<=0), then sqrt(-scr) with accumulate
                nc.vector.scalar_tensor_tensor(
                    out=scr[:], in0=nd[:], scalar=0.0, in1=mask[:],
                    op0=OP.min, op1=OP.mult)
                nc.scalar.activation(scr[:], scr[:], AF.Sqrt, scale=-1.0,
                                     accum_out=radii[:, rb : rb + 1])

                # transpose mask strip -> MT [j, i] for this row block
                mt = mtpool.tile([128, N], f32)
                for g in range(8):
                    tp = ps_mt.tile([128, 512], f32)
                    for kk in range(4):
                        c0 = (4 * g + kk) * 128
                        nc.tensor.transpose(tp[:, kk * 128:(kk + 1) * 128],
                                            mask[:, c0:c0 + 128], ident[:])
                    nc.scalar.activation(mt[:, g * 512:(g + 1) * 512], tp[:], AF.Copy)

                sps = ps_s.tile([128, 9], f32)
                for jb in range(NB):
                    nc.tensor.matmul(sps[:], mt[:, jb * 128:(jb + 1) * 128],
                                     phi[:, jb : jb + 1, :].squeeze(1), start=(jb == 0), stop=(jb == NB - 1))
                nc.scalar.activation(Sall[:, :, rb : rb + 1].squeeze(2), sps[:], AF.Copy)

        # ================= covariance + Jacobi + invariants =================
        jp = ctx.enter_context(tc.tile_pool(name="jacobi", bufs=1))

        _jtc = [0]

        def jt():
            _jtc[0] += 1
            return jp.tile([128, NB], f32, name=f"jt{_jtc[0]}")

        cx, cy, cz = (_sl(coords_nat, q) for q in range(3))
        inv_k = 1.0 / K

        # neighbor means
        m1 = [jp.tile([128, NB], f32, name=f"m1_{q}") for q in range(3)]
        for q in range(3):
            nc.scalar.activation(m1[q][:], Sall[:, q : q + 1, :].squeeze(1), AF.Copy, scale=inv_k)

        # cov components: S2ab/K - ca*m1b - cb*m1a + ca*cb
        cvs = [cx, cy, cz]
        A = {}
        for qi, (a, b) in enumerate(pq):
            t0 = jt()
            nc.scalar.activation(t0[:], Sall[:, 3 + qi : 4 + qi, :].squeeze(1), AF.Copy, scale=inv_k)
            t1 = jt()
            nc.vector.tensor_tensor(t1[:], cvs[a], m1[b][:], OP.mult)
            nc.vector.tensor_sub(t0[:], t0[:], t1[:])
            nc.vector.tensor_tensor(t1[:], cvs[b], m1[a][:], OP.mult)
            nc.vector.tensor_sub(t0[:], t0[:], t1[:])
            nc.vector.tensor_tensor(t1[:], cvs[a], cvs[b], OP.mult)
            nc.vector.tensor_add(t0[:], t0[:], t1[:])
            A[(a, b)] = t0

        # V = I (9 tiles)
        V = {}
        for i in range(3):
            for j in range(3):
                V[(i, j)] = jp.tile([128, NB], f32, name=f"V{i}{j}")
                nc.vector.memset(V[(i, j)][:], 1.0 if i == j else 0.0)

        def Ag(i, j):
            return A[(i, j)] if i <= j else A[(j, i)]

        tmp1, tmp2, tmp3, tmp4 = jt(), jt(), jt(), jt()
        SWEEPS = 4
        for sw in range(SWEEPS):
            for (p, q) in ((0, 1), (0, 2), (1, 2)):
                app, aqq, apq = Ag(p, p), Ag(q, q), Ag(p, q)
                # theta = (aqq - app) / (2*apq), guarded; t = sgn/( |th| + sqrt(th^2+1) )
                sA = tmp1
                nc.vector.tensor_scalar(sA[:], apq[:], 0.0, None, op0=OP.is_ge)
                nc.vector.tensor_scalar(sA[:], sA[:], 2.0, -1.0, op0=OP.mult, op1=OP.add)
                den = tmp2
                nc.vector.tensor_tensor(den[:], apq[:], sA[:], OP.mult)  # |apq|
                nc.vector.tensor_scalar(den[:], den[:], 1e-30, 2.0, op0=OP.max, op1=OP.mult)
                nc.vector.reciprocal(den[:], den[:])                     # 1/(2|apq|)
                th = tmp3
                nc.vector.tensor_sub(th[:], app[:], aqq[:])
                nc.vector.tensor_tensor(th[:], th[:], den[:], OP.mult)
                nc.vector.tensor_tensor(th[:], th[:], sA[:], OP.mult)    # theta (signed)
                nc.vector.tensor_scalar(th[:], th[:], 1e17, -1e17, op0=OP.min, op1=OP.max)
                sT = tmp1
                nc.vector.tensor_scalar(sT[:], th[:], 0.0, None, op0=OP.is_ge)
                nc.vector.tensor_scalar(sT[:], sT[:], 2.0, -1.0, op0=OP.mult, op1=OP.add)
                tha = tmp2
                nc.vector.tensor_tensor(tha[:], th[:], sT[:], OP.mult)   # |theta|
                r_ = tmp4
                nc.vector.tensor_tensor(r_[:], th[:], th[:], OP.mult)
                nc.scalar.activation(r_[:], r_[:], AF.Sqrt, bias=1.0)    # sqrt(th^2+1)
                nc.vector.tensor_add(r_[:], r_[:], tha[:])
                nc.vector.reciprocal(r_[:], r_[:])
                tt = tmp2
                nc.vector.tensor_tensor(tt[:], r_[:], sT[:], OP.mult)    # tan(phi)
                cc_ = tmp3
                nc.vector.tensor_tensor(cc_[:], tt[:], tt[:], OP.mult)
                nc.scalar.activation(cc_[:], cc_[:], AF.Sqrt, bias=1.0)
                nc.vector.reciprocal(cc_[:], cc_[:])                     # cos(phi)
                ss_ = tmp4
                nc.vector.tensor_tensor(ss_[:], tt[:], cc_[:], OP.mult)  # sin(phi)
                cs, sn = cc_, ss_

                # update A
                c2, s2, csn = jt(), jt(), jt()
                nc.vector.tensor_tensor(c2[:], cs[:], cs[:], OP.mult)
                nc.vector.tensor_tensor(s2[:], sn[:], sn[:], OP.mult)
                nc.vector.tensor_tensor(csn[:], cs[:], sn[:], OP.mult)
                u1, u2, u3, u4 = jt(), jt(), jt(), jt()
                nc.vector.tensor_tensor(u1[:], c2[:], app[:], OP.mult)
                nc.vector.tensor_tensor(u2[:], csn[:], apq[:], OP.mult)
                nc.vector.tensor_tensor(u3[:], s2[:], aqq[:], OP.mult)
                nc.vector.tensor_tensor(u4[:], s2[:], app[:], OP.mult)
                nc.gpsimd.tensor_tensor(app[:], c2[:], aqq[:], OP.mult)  # app <- c2*aqq (tmp)
                # app' = u1 + 2*u2 + u3 ; aqq' = u4 - 2*u2 + c2*aqq
                nc.vector.scalar_tensor_tensor(u1[:], u2[:], 2.0, u1[:], op0=OP.mult, op1=OP.add)
                nc.vector.tensor_add(u1[:], u1[:], u3[:])
                nc.vector.scalar_tensor_tensor(u4[:], u2[:], -2.0, u4[:], op0=OP.mult, op1=OP.add)
                nc.vector.tensor_add(aqq[:], u4[:], app[:])
                nc.vector.tensor_copy(app[:], u1[:])
                nc.vector.memset(apq[:], 0.0)
                rr = 3 - p - q
                apr, aqr = Ag(p, rr), Ag(q, rr)
                nc.vector.tensor_tensor(u1[:], cs[:], apr[:], OP.mult)
                nc.vector.tensor_tensor(u2[:], sn[:], aqr[:], OP.mult)
                nc.vector.tensor_tensor(u3[:], sn[:], apr[:], OP.mult)
                nc.vector.tensor_tensor(u4[:], cs[:], aqr[:], OP.mult)
                nc.vector.tensor_add(apr[:], u1[:], u2[:])
                nc.vector.tensor_sub(aqr[:], u4[:], u3[:])
                # update V columns p,q (rows 0..2) on gpsimd
                for i in range(3):
                    vip, viq = V[(i, p)], V[(i, q)]
                    nc.gpsimd.tensor_tensor(u1[:], cs[:], vip[:], OP.mult)
                    nc.gpsimd.tensor_tensor(u2[:], sn[:], viq[:], OP.mult)
                    nc.gpsimd.tensor_tensor(u3[:], sn[:], vip[:], OP.mult)
                    nc.gpsimd.tensor_tensor(u4[:], cs[:], viq[:], OP.mult)
                    nc.gpsimd.tensor_add(vip[:], u1[:], u2[:])
                    nc.gpsimd.tensor_sub(viq[:], u4[:], u3[:])

        e0, e1, e2 = A[(0, 0)], A[(1, 1)], A[(2, 2)]
        smin, smid, smax = jt(), jt(), jt()
        nc.vector.tensor_tensor(smin[:], e0[:], e1[:], OP.min)
        nc.vector.tensor_tensor(smin[:], smin[:], e2[:], OP.min)
        nc.vector.tensor_tensor(smax[:], e0[:], e1[:], OP.max)
        nc.vector.tensor_tensor(smax[:], smax[:], e2[:], OP.max)
        nc.vector.tensor_add(smid[:], e0[:], e1[:])
        nc.vector.tensor_add(smid[:], smid[:], e2[:])
        esum = jt()
        nc.vector.tensor_copy(esum[:], smid[:])  # e0+e1+e2
        nc.vector.tensor_sub(smid[:], smid[:], smin[:])
        nc.vector.tensor_sub(smid[:], smid[:], smax[:])

        # select eigenvector of smallest eigenvalue: exclusive masks
        m0, m1m, m2m = jt(), jt(), jt()
        nc.vector.tensor_tensor(tmp1[:], e0[:], e1[:], OP.is_le)
        nc.vector.tensor_tensor(tmp2[:], e0[:], e2[:], OP.is_le)
        nc.vector.tensor_tensor(m0[:], tmp1[:], tmp2[:], OP.mult)
        nc.vector.tensor_tensor(tmp1[:], e1[:], e0[:], OP.is_lt)
        nc.vector.tensor_tensor(tmp2[:], e1[:], e2[:], OP.is_le)
        nc.vector.tensor_tensor(m1m[:], tmp1[:], tmp2[:], OP.mult)
        nc.vector.tensor_add(m2m[:], m0[:], m1m[:])
        nc.vector.tensor_scalar(m2m[:], m2m[:], -1.0, 1.0, op0=OP.mult, op1=OP.add)

        nrm = [jp.tile([128, NB], f32, name=f"nrm{q}") for q in range(3)]  # normals nx,ny,nz
        for i in range(3):
            nc.vector.tensor_tensor(tmp1[:], m0[:], V[(i, 0)][:], OP.mult)
            nc.vector.tensor_tensor(tmp2[:], m1m[:], V[(i, 1)][:], OP.mult)
            nc.vector.tensor_add(tmp1[:], tmp1[:], tmp2[:])
            nc.vector.tensor_tensor(tmp2[:], m2m[:], V[(i, 2)][:], OP.mult)
            nc.vector.tensor_add(nrm[i][:], tmp1[:], tmp2[:])

        # center + outward + orientation
        ones128 = jp.tile([128, 1], f32)
        nc.vector.memset(ones128[:], 1.0)
        partials = jp.tile([128, 3], f32)
        for q, cv in enumerate(cvs):
            nc.vector.tensor_reduce(partials[:, q : q + 1], cv, axis=mybir.AxisListType.X,
                                    op=OP.add)
        cps = psum_small.tile([1, 3], f32, name="tp")
        nc.tensor.matmul(cps[:], ones128[:], partials[:], start=True, stop=True)
        center1 = jp.tile([1, 3], f32)
        nc.scalar.activation(center1[:], cps[:], AF.Copy, scale=1.0 / N)
        center = jp.tile([128, 3], f32)
        cbps = psum_small.tile([128, 3], f32, name="tp")
        nc.tensor.matmul(cbps[:], ones_row[:], center1[:], start=True, stop=True)
        nc.scalar.activation(center[:], cbps[:], AF.Copy)

        ox, oy, oz = jt(), jt(), jt()
        ovs = [ox, oy, oz]
        for q in range(3):
            nc.vector.tensor_scalar(ovs[q][:], cvs[q], center[:, q : q + 1], None,
                                    op0=OP.subtract)
        dt_ = jt()
        nc.vector.tensor_tensor(dt_[:], nrm[0][:], ox[:], OP.mult)
        nc.vector.tensor_tensor(tmp1[:], nrm[1][:], oy[:], OP.mult)
        nc.vector.tensor_add(dt_[:], dt_[:], tmp1[:])
        nc.vector.tensor_tensor(tmp1[:], nrm[2][:], oz[:], OP.mult)
        nc.vector.tensor_add(dt_[:], dt_[:], tmp1[:])
        nc.vector.tensor_scalar(dt_[:], dt_[:], 0.0, None, op0=OP.is_ge)
        nc.vector.tensor_scalar(dt_[:], dt_[:], 2.0, -1.0, op0=OP.mult, op1=OP.add)
        for i in range(3):
            nc.vector.tensor_tensor(nrm[i][:], nrm[i][:], dt_[:], OP.mult)
        # normalize
        nc.vector.tensor_tensor(tmp1[:], nrm[0][:], nrm[0][:], OP.mult)
        nc.vector.tensor_tensor(tmp2[:], nrm[1][:], nrm[1][:], OP.mult)
        nc.vector.tensor_add(tmp1[:], tmp1[:], tmp2[:])
        nc.vector.tensor_tensor(tmp2[:], nrm[2][:], nrm[2][:], OP.mult)
        nc.vector.tensor_add(tmp1[:], tmp1[:], tmp2[:])
        nc.scalar.activation(tmp1[:], tmp1[:], AF.Sqrt)
        nc.vector.tensor_scalar(tmp1[:], tmp1[:], 1e-6, None, op0=OP.max)
        nc.vector.reciprocal(tmp1[:], tmp1[:])
        for i in range(3):
            nc.vector.tensor_tensor(nrm[i][:], nrm[i][:], tmp1[:], OP.mult)

        # centered radius + dominance + plane offset
        cr = jt()
        nc.vector.tensor_tensor(cr[:], ox[:], ox[:], OP.mult)
        nc.vector.tensor_tensor(tmp1[:], oy[:], oy[:], OP.mult)
        nc.vector.tensor_add(cr[:], cr[:], tmp1[:])
        nc.vector.tensor_tensor(tmp1[:], oz[:], oz[:], OP.mult)
        nc.vector.tensor_add(cr[:], cr[:], tmp1[:])
        nc.scalar.activation(cr[:], cr[:], AF.Sqrt)
        dom = jt()
        nc.vector.tensor_scalar(tmp1[:], esum[:], 1e-6, None, op0=OP.max)
        nc.vector.reciprocal(tmp1[:], tmp1[:])
        nc.vector.tensor_tensor(dom[:], smax[:], tmp1[:], OP.mult)
        pd = jt()  # + sum(c*n); negated at assembly
        nc.vector.tensor_tensor(pd[:], cx, nrm[0][:], OP.mult)
        nc.vector.tensor_tensor(tmp1[:], cy, nrm[1][:], OP.mult)
        nc.vector.tensor_add(pd[:], pd[:], tmp1[:])
        nc.vector.tensor_tensor(tmp1[:], cz, nrm[2][:], OP.mult)
        nc.vector.tensor_add(pd[:], pd[:], tmp1[:])

        # ================= MLPs (transposed activations) =================
        inv_stage = jp.tile([128, NB, 6], f32)
        nc.vector.tensor_copy(_sl(inv_stage, 0), smin[:])
        nc.vector.tensor_copy(_sl(inv_stage, 1), smid[:])
        nc.vector.tensor_copy(_sl(inv_stage, 2), smax[:])
        nc.scalar.activation(_sl(inv_stage, 3), radii[:], AF.Copy, scale=1.0 / K)
        nc.vector.tensor_copy(_sl(inv_stage, 4), cr[:])
        nc.vector.tensor_copy(_sl(inv_stage, 5), dom[:])

        invT = jp.tile([6, N], f32)
        for rb in range(NB):
            tp = psum_small.tile([6, 128], f32, name="tp")
            nc.tensor.transpose(tp[:], inv_stage[:, rb : rb + 1, :].squeeze(1), ident[:])
            nc.scalar.activation(invT[:, rb * 128:(rb + 1) * 128], tp[:], AF.Copy)

        hidden = jp.tile([128, N], f32)
        outstage = jp.tile([128, NB, 16], f32)
        nc.vector.memset(outstage[:], 0.0)

        with (
            tc.tile_pool(name="mlp_sb", bufs=3) as mlp_sb,
            tc.tile_pool(name="ps_mlp", bufs=4, space="PSUM") as ps_mlp,
        ):
            for ch in range(8):
                cslice = slice(ch * 512, (ch + 1) * 512)
                # inv branch
                p1 = ps_mlp.tile([128, 512], f32, name="pmlp")
                nc.tensor.matmul(p1[:], w["inv_W1"][:],
                                 invT[:, cslice], start=True, stop=True)
                h1 = mlp_sb.tile([128, 512], f32)
                nc.scalar.activation(h1[:], p1[:], AF.Gelu, bias=w["inv_b1"][:])
                p2 = ps_mlp.tile([128, 512], f32, name="pmlp")
                nc.tensor.matmul(p2[:], w["inv_W2"][:],
                                 h1[:], start=True, stop=True)
                ih = mlp_sb.tile([128, 512], f32)
                nc.scalar.activation(ih[:], p2[:], AF.Identity, bias=w["inv_b2"][:])
                # feat branch
                p3 = ps_mlp.tile([128, 512], f32, name="pmlp")
                nc.tensor.matmul(p3[:], w["feat_W1"][:],
                                 featT[:, cslice], start=True, stop=True)
                h2 = mlp_sb.tile([128, 512], f32)
                nc.scalar.activation(h2[:], p3[:], AF.Gelu, bias=w["feat_b1"][:])
                p4 = ps_mlp.tile([128, 512], f32, name="pmlp")
                nc.tensor.matmul(p4[:], w["feat_W2"][:],
                                 h2[:], start=True, stop=True)
                fh = mlp_sb.tile([128, 512], f32)
                nc.scalar.activation(fh[:], p4[:], AF.Identity, bias=w["feat_b2"][:])
                # shared
                p5 = ps_mlp.tile([128, 512], f32, name="pmlp")
                nc.tensor.matmul(p5[:], w["sh_W1a"][:], ih[:],
                                 start=True, stop=False)
                nc.tensor.matmul(p5[:], w["sh_W1b"][:], fh[:],
                                 start=False, stop=True)
                hs = mlp_sb.tile([128, 512], f32)
                nc.scalar.activation(hs[:], p5[:], AF.Gelu, bias=w["sh_b1"][:])
                p6 = ps_mlp.tile([128, 512], f32, name="pmlp")
                nc.tensor.matmul(p6[:], w["sh_W2"][:], hs[:],
                                 start=True, stop=True)
                nc.scalar.activation(hidden[:, cslice], p6[:], AF.Identity,
                                     bias=w["sh_b2"][:])

            # scalar head: per row block, lhsT = hidden strip, rhs = g0_W
            for rb in range(NB):
                sp = ps_mlp.tile([128, 1], f32, name="psp", bufs=2)
                nc.tensor.matmul(sp[:], hidden[:, rb * 128:(rb + 1) * 128], w["g0_W"][:],
                                 start=True, stop=True)
                nc.scalar.activation(outstage[:, rb : rb + 1, 0:1].squeeze(1), sp[:], AF.Identity,
                                     bias=w["g0_b"][:])

        # ================= assembly + output DMA =================
        nc.vector.tensor_copy(_sl(outstage, 1), nrm[0][:])
        nc.vector.tensor_copy(_sl(outstage, 2), nrm[1][:])
        nc.scalar.activation(_sl(outstage, 3), nrm[2][:], AF.Copy, scale=-1.0)
        nc.scalar.activation(_sl(outstage, 4), pd[:], AF.Copy, scale=-1.0)
        nc.vector.tensor_copy(_sl(outstage, 5), nrm[0][:])
        nc.vector.tensor_copy(_sl(outstage, 6), nrm[1][:])
        nc.vector.tensor_copy(_sl(outstage, 7), nrm[2][:])
        nc.vector.tensor_copy(_sl(outstage, 11), cx)
        nc.vector.tensor_copy(_sl(outstage, 12), cy)
        nc.vector.tensor_copy(_sl(outstage, 13), cz)
        nc.vector.memset(_sl(outstage, 14), 1.0)

        for rb in range(NB):
            nc.sync.dma_start(out_h[rb * 128:(rb + 1) * 128, :], outstage[:, rb : rb + 1, :].squeeze(1))


# ---------------------------------------------------------------------------
# SPMD driver: batch b -> core b (persistent PJRT executable, axon-tunneled)
# ---------------------------------------------------------------------------
_WKEYS = ("inv_W1", "inv_b1", "inv_W2", "inv_b2", "feat_W1", "feat_b1",
          "feat_W2", "feat_b2", "sh_W1", "sh_b1", "sh_W2", "sh_b2",
          "g0_W", "g0_b")
_B = 8
_CACHE = {}


def _get_runner():
    if "runner" in _CACHE:
        return _CACHE["runner"]
    import jax
    from jax.sharding import Mesh, PartitionSpec
    from jax.experimental.shard_map import shard_map
    from concourse import bass2jax, mybir as _mb
    from concourse.bass2jax import _bass_exec_p, install_neuronx_cc_hook

    install_neuronx_cc_hook()
    nc = build_nc()
    partition_name = nc.partition_id_tensor.name if nc.partition_id_tensor else None
    in_names, out_names, out_avals = [], [], []
    for alloc in nc.m.functions[0].allocations:
        if not isinstance(alloc, _mb.MemoryLocationSet):
            continue
        name = alloc.memorylocations[0].name
        if alloc.kind == "ExternalInput":
            if name != partition_name:
                in_names.append(name)
        elif alloc.kind == "ExternalOutput":
            out_avals.append(jax.core.ShapedArray(tuple(alloc.tensor_shape),
                                                  _mb.dt.np(alloc.dtype)))
            out_names.append(name)
    n_params = len(in_names)
    n_outs = len(out_avals)
    all_in = list(in_names) + list(out_names)
    if partition_name is not None:
        all_in.append(partition_name)

    def _body(*args):
        operands = list(args)
        if partition_name is not None:
            operands.append(bass2jax.partition_id_tensor())
        return tuple(_bass_exec_p.bind(
            *operands, out_avals=tuple(out_avals), in_names=tuple(all_in),
            out_names=tuple(out_names), lowering_input_output_aliases=(),
            sim_require_finite=True, sim_require_nnan=True, nc=nc))

    devices = jax.devices()[:_B]
    mesh = Mesh(np.asarray(devices), ("core",))
    sharded = jax.jit(
        shard_map(_body, mesh=mesh,
                  in_specs=(PartitionSpec("core"),) * (n_params + n_outs),
                  out_specs=(PartitionSpec("core"),) * n_outs,
                  check_rep=False),
        keep_unused=True)
    zeros_dev = [jax.device_put(
        np.zeros((_B * a.shape[0],) + tuple(a.shape[1:]), a.dtype))
        for a in out_avals]
    _CACHE["runner"] = (sharded, in_names, out_names, out_avals, zeros_dev)
    return _CACHE["runner"]


def _concat_inputs(inputs, in_names):
    per_core = []
    for b in range(_B):
        m = {
            "coords": np.ascontiguousarray(inputs["coords"][b], dtype=np.float32),
            "features": np.ascontiguousarray(inputs["features"][b], dtype=np.float32),
        }
        for k in _WKEYS:
            m[k] = np.ascontiguousarray(np.asarray(inputs[k], dtype=np.float32))
        per_core.append([m[n] for n in in_names])
    return [np.concatenate([per_core[c][i] for c in range(_B)], axis=0)
            for i in range(len(in_names))]


def run(inputs):
    sharded, in_names, out_names, out_avals, zeros_dev = _get_runner()
    outs = sharded(*_concat_inputs(inputs, in_names), *zeros_dev)
    i = out_names.index("out")
    return np.asarray(outs[i]).reshape(_B, *out_avals[i].shape).astype(np.float32)


def time_kernel(inputs, iters=20):
    """Steady-state per-invocation wall time (ns), pipelined dispatch."""
    import time as _time
    import jax
    sharded, in_names, out_names, out_avals, zeros_dev = _get_runner()
    dev_in = [jax.device_put(a) for a in _concat_inputs(inputs, in_names)]
    for _ in range(2):  # warm
        jax.block_until_ready(sharded(*dev_in, *zeros_dev))
    t0 = _time.perf_counter()
    outs = [sharded(*dev_in, *zeros_dev) for _ in range(iters)]
    jax.block_until_ready(outs)
    return (_time.perf_counter() - t0) / iters * 1e9


def kernel(**inputs):
    return run(inputs)


# revision 6
# speedup vs baseline: 13.5418x; 1.0909x over previous
"""Bass/Tile kernel for nn_PointCloudMPE: per-core pipeline for one batch
(coords [4096,3], features [4096,64]) -> out [4096,16].

Pipeline: pairwise -dist^2 via PE matmul (homogeneous 5-dim trick) ->
top-16 threshold via DVE max8/match_replace/max8 -> 0/1 neighbor mask ->
masked moment sums via PE matmuls (mask transposed on PE) -> 3x3 covariance
-> batched branch-free cyclic Jacobi eigensolver -> normals/invariants ->
MLPs on PE (transposed activations) -> assembled [4096,16] output.
"""

import numpy as np

import concourse.bass as bass
import concourse.bacc as bacc
import concourse.mybir as mybir
import concourse.tile as tile
from concourse.masks import make_identity

f32 = mybir.dt.float32
f32r = mybir.dt.float32r
AF = mybir.ActivationFunctionType
OP = mybir.AluOpType

N, K, NB = 4096, 16, 32  # points, knn, row blocks of 128
NEG_BIG = -1.0e30


def _sl(t, q):
    # [128, 32, C] tile -> [128, 32] strided view of component q
    return t[:, :, q : q + 1].squeeze(2)


def build_nc():
    nc = bacc.Bacc("TRN2", target_bir_lowering=False)

    coords_h = nc.declare_dram_parameter("coords", [N, 3], f32, isOutput=False)
    feats_h = nc.declare_dram_parameter("features", [N, 64], f32, isOutput=False)
    wnames = {
        "inv_W1": [6, 128], "inv_b1": [128], "inv_W2": [128, 128], "inv_b2": [128],
        "feat_W1": [64, 128], "feat_b1": [128], "feat_W2": [128, 128], "feat_b2": [128],
        "sh_W1": [256, 128], "sh_b1": [128], "sh_W2": [128, 128], "sh_b2": [128],
        "g0_W": [128, 1], "g0_b": [1],
    }
    wh = {k: nc.declare_dram_parameter(k, shp, f32, isOutput=False)
          for k, shp in wnames.items()}
    out_h = nc.declare_dram_parameter("out", [N, 16], f32, isOutput=True)

    with tile.TileContext(nc) as tc:
        _body(nc, tc, coords_h, feats_h, wh, out_h)
    nc.finalize()
    return nc


def _body(nc, tc, coords_h, feats_h, wh, out_h):
    import contextlib

    ctx = contextlib.ExitStack()
    with ctx:
        persist = ctx.enter_context(tc.tile_pool(name="persist", bufs=1))
        psum_small = ctx.enter_context(tc.tile_pool(name="ps_small", bufs=2, space="PSUM"))

        ident = persist.tile([128, 128], f32)
        make_identity(nc, ident[:])

        # ---- load coords in block layout + weights ----
        coords_nat = persist.tile([128, NB, 3], f32)
        nc.sync.dma_start(coords_nat[:],
                          coords_h[:, :].rearrange("(rb p) d -> p rb d", p=128))

        w = {}
        for k in ("inv_W1", "inv_W2", "feat_W1", "feat_W2", "sh_W2", "g0_W"):
            shp = [wh[k].shape[0], wh[k].shape[1]]
            w[k] = persist.tile(shp, f32, name=f"w_{k}")
            nc.sync.dma_start(w[k][:], wh[k][:, :])
        w["sh_W1a"] = persist.tile([128, 128], f32, name="w_sh_W1a")
        w["sh_W1b"] = persist.tile([128, 128], f32, name="w_sh_W1b")
        nc.sync.dma_start(w["sh_W1a"][:], wh["sh_W1"][0:128, :])
        nc.sync.dma_start(w["sh_W1b"][:], wh["sh_W1"][128:256, :])
        for k in ("inv_b1", "inv_b2", "feat_b1", "feat_b2", "sh_b1", "sh_b2"):
            w[k] = persist.tile([128, 1], f32, name=f"w_{k}")
            nc.sync.dma_start(w[k][:], wh[k][:].unsqueeze(1))
        g0b_one = persist.tile([1, 1], f32)
        nc.sync.dma_start(g0b_one[:], wh["g0_b"][:].unsqueeze(1))
        ones_row = persist.tile([1, 128], f32)
        nc.vector.memset(ones_row[:], 1.0)
        w["g0_b"] = persist.tile([128, 1], f32, name="w_g0_b_bc")
        g0ps = psum_small.tile([128, 1], f32, name="tp")
        nc.tensor.matmul(g0ps[:], ones_row[:], g0b_one[:], start=True, stop=True)
        nc.scalar.activation(w["g0_b"][:], g0ps[:], AF.Copy)

        # ---- phi [128, 32, 9]: x,y,z,xx,xy,xz,yy,yz,zz per point ----
        phi = persist.tile([128, NB, 9], f32)
        nc.vector.tensor_copy(phi[:, :, 0:3], coords_nat[:])
        pq = [(0, 0), (0, 1), (0, 2), (1, 1), (1, 2), (2, 2)]
        for qi, (a, b) in enumerate(pq):
            nc.vector.tensor_tensor(
                phi[:, :, 3 + qi : 4 + qi], coords_nat[:, :, a : a + 1],
                coords_nat[:, :, b : b + 1], OP.mult)

        # ---- L/R matrices for -dist^2 matmul, via block-layout staging ----
        # L col j: (x,y,z,sq,1); R col j: (2x,2y,2z,-1,-sq)
        sq_nat = persist.tile([128, NB], f32)
        nc.vector.tensor_add(sq_nat[:], _sl(phi, 3), _sl(phi, 6))
        nc.vector.tensor_add(sq_nat[:], sq_nat[:], _sl(phi, 8))
        Lstage = persist.tile([128, NB, 5], f32)
        Rstage = persist.tile([128, NB, 5], f32)
        nc.vector.tensor_copy(Lstage[:, :, 0:3], coords_nat[:])
        nc.vector.tensor_copy(_sl(Lstage, 3), sq_nat[:])
        nc.vector.memset(_sl(Lstage, 4), 1.0)
        nc.scalar.activation(Rstage[:, :, 0:3], coords_nat[:], AF.Copy, scale=2.0)
        nc.vector.memset(_sl(Rstage, 3), -1.0)
        nc.scalar.activation(_sl(Rstage, 4), sq_nat[:], AF.Copy, scale=-1.0)

        L_all = persist.tile([5, N], f32)
        R_all = persist.tile([5, N], f32)
        for rb in range(NB):
            tpl = psum_small.tile([5, 128], f32, name="tp")
            nc.tensor.transpose(tpl[:], Lstage[:, rb : rb + 1, :].squeeze(1), ident[:])
            nc.scalar.activation(L_all[:, rb * 128:(rb + 1) * 128], tpl[:], AF.Copy)
            tpr = psum_small.tile([5, 128], f32, name="tp")
            nc.tensor.transpose(tpr[:], Rstage[:, rb : rb + 1, :].squeeze(1), ident[:])
            nc.scalar.activation(R_all[:, rb * 128:(rb + 1) * 128], tpr[:], AF.Copy)

        # ---- featT [64, 4096] ----
        featT = persist.tile([64, N], f32)
        with tc.tile_pool(name="ftile", bufs=2) as fpool:
            for rb in range(NB):
                ft = fpool.tile([128, 64], f32)
                nc.sync.dma_start(ft[:], feats_h[rb * 128:(rb + 1) * 128, :])
                tp = psum_small.tile([64, 128], f32, name="tp")
                nc.tensor.transpose(tp[:], ft[:], ident[:])
                nc.scalar.activation(featT[:, rb * 128:(rb + 1) * 128], tp[:], AF.Copy)

        # ---- per-point accumulators (block layout [128, 32]) ----
        radii = persist.tile([128, NB], f32)
        Sall = persist.tile([128, 9, NB], f32)

        # ---- main selection + moment loop over row blocks ----
        with (
            tc.tile_pool(name="ndpool", bufs=2) as ndpool,
            tc.tile_pool(name="scrpool", bufs=2) as scrpool,
            tc.tile_pool(name="mpool", bufs=2) as mpool,
            tc.tile_pool(name="mtpool", bufs=2) as mtpool,
            tc.tile_pool(name="selpool", bufs=2) as selpool,
            tc.tile_pool(name="ps_nd", bufs=2, space="PSUM") as ps_nd,
            tc.tile_pool(name="ps_mt", bufs=2, space="PSUM") as ps_mt,
            tc.tile_pool(name="ps_s", bufs=2, space="PSUM") as ps_s,
        ):
            def compute_nd(rb):
                nd = ndpool.tile([128, N], f32, name="nd")
                lhs = L_all[:, rb * 128:(rb + 1) * 128]
                for ct in range(8):
                    ps = ps_nd.tile([128, 512], f32, name="ps")
                    nc.tensor.matmul(ps[:], lhs, R_all[:, ct * 512:(ct + 1) * 512],
                                     start=True, stop=True)
                    nc.scalar.activation(nd[:, ct * 512:(ct + 1) * 512], ps[:], AF.Copy)
                # self-exclusion: diagonal block columns rb*128..rb*128+127
                nc.gpsimd.affine_select(
                    out=nd[:, rb * 128:(rb + 1) * 128],
                    in_=nd[:, rb * 128:(rb + 1) * 128],
                    compare_op=OP.not_equal, fill=NEG_BIG,
                    base=0, pattern=[[1, 128]], channel_multiplier=-1)
                return nd

            nd_next = compute_nd(0)
            for rb in range(NB):
                nd = nd_next
                if rb + 1 < NB:
                    nd_next = compute_nd(rb + 1)

                mx1 = selpool.tile([128, 8], f32)
                mx2 = selpool.tile([128, 8], f32)
                scr = scrpool.tile([128, N], f32)
                nc.vector.max(out=mx1[:], in_=nd[:])
                nc.vector.match_replace(out=scr[:], in_to_replace=mx1[:],
                                        in_values=nd[:], imm_value=NEG_BIG)
                nc.vector.max(out=mx2[:], in_=scr[:])

                mask = mpool.tile([128, N], f32)
                nc.gpsimd.tensor_scalar(mask[:], nd[:], mx2[:, 7:8], None, op0=OP.is_ge)

                # radius: the 16 selected -dist^2 values are mx1/mx2 [128,8];
                # sum sqrt(dist) via two tiny clamped sqrts with accumulate
                mc1 = selpool.tile([128, 8], f32, name="mc1")
                mc2 = selpool.tile([128, 8], f32, name="mc2")
                nc.vector.tensor_scalar(mc1[:], mx1[:], 0.0, None, op0=OP.min)
                nc.vector.tensor_scalar(mc2[:], mx2[:], 0.0, None, op0=OP.min)
                r1 = selpool.tile([128, 1], f32, name="r1")
                r2 = selpool.tile([128, 1], f32, name="r2")
                nc.scalar.activation(mc1[:], mc1[:], AF.Sqrt, scale=-1.0,
                                     accum_out=r1[:])
                nc.scalar.activation(mc2[:], mc2[:], AF.Sqrt, scale=-1.0,
                                     accum_out=r2[:])
                nc.vector.tensor_add(radii[:, rb : rb + 1], r1[:], r2[:])

                # transpose mask strip -> MT [j, i] for this row block
                mt = mtpool.tile([128, N], f32)
                for g in range(8):
                    tp = ps_mt.tile([128, 512], f32)
                    for kk in range(4):
                        c0 = (4 * g + kk) * 128
                        nc.tensor.transpose(tp[:, kk * 128:(kk + 1) * 128],
                                            mask[:, c0:c0 + 128], ident[:])
                    nc.scalar.activation(mt[:, g * 512:(g + 1) * 512], tp[:], AF.Copy)

                sps = ps_s.tile([128, 9], f32)
                for jb in range(NB):
                    nc.tensor.matmul(sps[:], mt[:, jb * 128:(jb + 1) * 128],
                                     phi[:, jb : jb + 1, :].squeeze(1), start=(jb == 0), stop=(jb == NB - 1))
                nc.scalar.activation(Sall[:, :, rb : rb + 1].squeeze(2), sps[:], AF.Copy)

        # ================= covariance + Jacobi + invariants =================
        jp = ctx.enter_context(tc.tile_pool(name="jacobi", bufs=1))

        _jtc = [0]

        def jt():
            _jtc[0] += 1
            return jp.tile([128, NB], f32, name=f"jt{_jtc[0]}")

        cx, cy, cz = (_sl(coords_nat, q) for q in range(3))
        inv_k = 1.0 / K

        # neighbor means
        m1 = [jp.tile([128, NB], f32, name=f"m1_{q}") for q in range(3)]
        for q in range(3):
            nc.scalar.activation(m1[q][:], Sall[:, q : q + 1, :].squeeze(1), AF.Copy, scale=inv_k)

        # cov components: S2ab/K - ca*m1b - cb*m1a + ca*cb
        cvs = [cx, cy, cz]
        A = {}
        for qi, (a, b) in enumerate(pq):
            t0 = jt()
            nc.scalar.activation(t0[:], Sall[:, 3 + qi : 4 + qi, :].squeeze(1), AF.Copy, scale=inv_k)
            t1 = jt()
            nc.vector.tensor_tensor(t1[:], cvs[a], m1[b][:], OP.mult)
            nc.vector.tensor_sub(t0[:], t0[:], t1[:])
            nc.vector.tensor_tensor(t1[:], cvs[b], m1[a][:], OP.mult)
            nc.vector.tensor_sub(t0[:], t0[:], t1[:])
            nc.vector.tensor_tensor(t1[:], cvs[a], cvs[b], OP.mult)
            nc.vector.tensor_add(t0[:], t0[:], t1[:])
            A[(a, b)] = t0

        # V = I (9 tiles)
        V = {}
        for i in range(3):
            for j in range(3):
                V[(i, j)] = jp.tile([128, NB], f32, name=f"V{i}{j}")
                nc.vector.memset(V[(i, j)][:], 1.0 if i == j else 0.0)

        def Ag(i, j):
            return A[(i, j)] if i <= j else A[(j, i)]

        tmp1, tmp2, tmp3, tmp4 = jt(), jt(), jt(), jt()
        SWEEPS = 4
        for sw in range(SWEEPS):
            for (p, q) in ((0, 1), (0, 2), (1, 2)):
                app, aqq, apq = Ag(p, p), Ag(q, q), Ag(p, q)
                # theta = (aqq - app) / (2*apq), guarded; t = sgn/( |th| + sqrt(th^2+1) )
                sA = tmp1
                nc.vector.tensor_scalar(sA[:], apq[:], 0.0, None, op0=OP.is_ge)
                nc.vector.tensor_scalar(sA[:], sA[:], 2.0, -1.0, op0=OP.mult, op1=OP.add)
                den = tmp2
                nc.vector.tensor_tensor(den[:], apq[:], sA[:], OP.mult)  # |apq|
                nc.vector.tensor_scalar(den[:], den[:], 1e-30, 2.0, op0=OP.max, op1=OP.mult)
                nc.vector.reciprocal(den[:], den[:])                     # 1/(2|apq|)
                th = tmp3
                nc.vector.tensor_sub(th[:], app[:], aqq[:])
                nc.vector.tensor_tensor(th[:], th[:], den[:], OP.mult)
                nc.vector.tensor_tensor(th[:], th[:], sA[:], OP.mult)    # theta (signed)
                nc.vector.tensor_scalar(th[:], th[:], 1e17, -1e17, op0=OP.min, op1=OP.max)
                sT = tmp1
                nc.vector.tensor_scalar(sT[:], th[:], 0.0, None, op0=OP.is_ge)
                nc.vector.tensor_scalar(sT[:], sT[:], 2.0, -1.0, op0=OP.mult, op1=OP.add)
                tha = tmp2
                nc.vector.tensor_tensor(tha[:], th[:], sT[:], OP.mult)   # |theta|
                r_ = tmp4
                nc.vector.tensor_tensor(r_[:], th[:], th[:], OP.mult)
                nc.scalar.activation(r_[:], r_[:], AF.Sqrt, bias=1.0)    # sqrt(th^2+1)
                nc.vector.tensor_add(r_[:], r_[:], tha[:])
                nc.vector.reciprocal(r_[:], r_[:])
                tt = tmp2
                nc.vector.tensor_tensor(tt[:], r_[:], sT[:], OP.mult)    # tan(phi)
                cc_ = tmp3
                nc.vector.tensor_tensor(cc_[:], tt[:], tt[:], OP.mult)
                nc.scalar.activation(cc_[:], cc_[:], AF.Sqrt, bias=1.0)
                nc.vector.reciprocal(cc_[:], cc_[:])                     # cos(phi)
                ss_ = tmp4
                nc.vector.tensor_tensor(ss_[:], tt[:], cc_[:], OP.mult)  # sin(phi)
                cs, sn = cc_, ss_

                # update A
                c2, s2, csn = jt(), jt(), jt()
                nc.vector.tensor_tensor(c2[:], cs[:], cs[:], OP.mult)
                nc.vector.tensor_tensor(s2[:], sn[:], sn[:], OP.mult)
                nc.vector.tensor_tensor(csn[:], cs[:], sn[:], OP.mult)
                u1, u2, u3, u4 = jt(), jt(), jt(), jt()
                nc.vector.tensor_tensor(u1[:], c2[:], app[:], OP.mult)
                nc.vector.tensor_tensor(u2[:], csn[:], apq[:], OP.mult)
                nc.vector.tensor_tensor(u3[:], s2[:], aqq[:], OP.mult)
                nc.vector.tensor_tensor(u4[:], s2[:], app[:], OP.mult)
                nc.gpsimd.tensor_tensor(app[:], c2[:], aqq[:], OP.mult)  # app <- c2*aqq (tmp)
                # app' = u1 + 2*u2 + u3 ; aqq' = u4 - 2*u2 + c2*aqq
                nc.vector.scalar_tensor_tensor(u1[:], u2[:], 2.0, u1[:], op0=OP.mult, op1=OP.add)
                nc.vector.tensor_add(u1[:], u1[:], u3[:])
                nc.vector.scalar_tensor_tensor(u4[:], u2[:], -2.0, u4[:], op0=OP.mult, op1=OP.add)
                nc.vector.tensor_add(aqq[:], u4[:], app[:])
                nc.vector.tensor_copy(app[:], u1[:])
                nc.vector.memset(apq[:], 0.0)
                rr = 3 - p - q
                apr, aqr = Ag(p, rr), Ag(q, rr)
                nc.vector.tensor_tensor(u1[:], cs[:], apr[:], OP.mult)
                nc.vector.tensor_tensor(u2[:], sn[:], aqr[:], OP.mult)
                nc.vector.tensor_tensor(u3[:], sn[:], apr[:], OP.mult)
                nc.vector.tensor_tensor(u4[:], cs[:], aqr[:], OP.mult)
                nc.vector.tensor_add(apr[:], u1[:], u2[:])
                nc.vector.tensor_sub(aqr[:], u4[:], u3[:])
                # update V columns p,q (rows 0..2) on gpsimd
                for i in range(3):
                    vip, viq = V[(i, p)], V[(i, q)]
                    nc.gpsimd.tensor_tensor(u1[:], cs[:], vip[:], OP.mult)
                    nc.gpsimd.tensor_tensor(u2[:], sn[:], viq[:], OP.mult)
                    nc.gpsimd.tensor_tensor(u3[:], sn[:], vip[:], OP.mult)
                    nc.gpsimd.tensor_tensor(u4[:], cs[:], viq[:], OP.mult)
                    nc.gpsimd.tensor_add(vip[:], u1[:], u2[:])
                    nc.gpsimd.tensor_sub(viq[:], u4[:], u3[:])

        e0, e1, e2 = A[(0, 0)], A[(1, 1)], A[(2, 2)]
        smin, smid, smax = jt(), jt(), jt()
        nc.vector.tensor_tensor(smin[:], e0[:], e1[:], OP.min)
        nc.vector.tensor_tensor(smin[:], smin[:], e2[:], OP.min)
        nc.vector.tensor_tensor(smax[:], e0[:], e1[:], OP.max)
        nc.vector.tensor_tensor(smax[:], smax[:], e2[:], OP.max)
        nc.vector.tensor_add(smid[:], e0[:], e1[:])
        nc.vector.tensor_add(smid[:], smid[:], e2[:])
        esum = jt()
        nc.vector.tensor_copy(esum[:], smid[:])  # e0+e1+e2
        nc.vector.tensor_sub(smid[:], smid[:], smin[:])
        nc.vector.tensor_sub(smid[:], smid[:], smax[:])

        # select eigenvector of smallest eigenvalue: exclusive masks
        m0, m1m, m2m = jt(), jt(), jt()
        nc.vector.tensor_tensor(tmp1[:], e0[:], e1[:], OP.is_le)
        nc.vector.tensor_tensor(tmp2[:], e0[:], e2[:], OP.is_le)
        nc.vector.tensor_tensor(m0[:], tmp1[:], tmp2[:], OP.mult)
        nc.vector.tensor_tensor(tmp1[:], e1[:], e0[:], OP.is_lt)
        nc.vector.tensor_tensor(tmp2[:], e1[:], e2[:], OP.is_le)
        nc.vector.tensor_tensor(m1m[:], tmp1[:], tmp2[:], OP.mult)
        nc.vector.tensor_add(m2m[:], m0[:], m1m[:])
        nc.vector.tensor_scalar(m2m[:], m2m[:], -1.0, 1.0, op0=OP.mult, op1=OP.add)

        nrm = [jp.tile([128, NB], f32, name=f"nrm{q}") for q in range(3)]  # normals nx,ny,nz
        for i in range(3):
            nc.vector.tensor_tensor(tmp1[:], m0[:], V[(i, 0)][:], OP.mult)
            nc.vector.tensor_tensor(tmp2[:], m1m[:], V[(i, 1)][:], OP.mult)
            nc.vector.tensor_add(tmp1[:], tmp1[:], tmp2[:])
            nc.vector.tensor_tensor(tmp2[:], m2m[:], V[(i, 2)][:], OP.mult)
            nc.vector.tensor_add(nrm[i][:], tmp1[:], tmp2[:])

        # center + outward + orientation
        ones128 = jp.tile([128, 1], f32)
        nc.vector.memset(ones128[:], 1.0)
        partials = jp.tile([128, 3], f32)
        for q, cv in enumerate(cvs):
            nc.vector.tensor_reduce(partials[:, q : q + 1], cv, axis=mybir.AxisListType.X,
                                    op=OP.add)
        cps = psum_small.tile([1, 3], f32, name="tp")
        nc.tensor.matmul(cps[:], ones128[:], partials[:], start=True, stop=True)
        center1 = jp.tile([1, 3], f32)
        nc.scalar.activation(center1[:], cps[:], AF.Copy, scale=1.0 / N)
        center = jp.tile([128, 3], f32)
        cbps = psum_small.tile([128, 3], f32, name="tp")
        nc.tensor.matmul(cbps[:], ones_row[:], center1[:], start=True, stop=True)
        nc.scalar.activation(center[:], cbps[:], AF.Copy)

        ox, oy, oz = jt(), jt(), jt()
        ovs = [ox, oy, oz]
        for q in range(3):
            nc.vector.tensor_scalar(ovs[q][:], cvs[q], center[:, q : q + 1], None,
                                    op0=OP.subtract)
        dt_ = jt()
        nc.vector.tensor_tensor(dt_[:], nrm[0][:], ox[:], OP.mult)
        nc.vector.tensor_tensor(tmp1[:], nrm[1][:], oy[:], OP.mult)
        nc.vector.tensor_add(dt_[:], dt_[:], tmp1[:])
        nc.vector.tensor_tensor(tmp1[:], nrm[2][:], oz[:], OP.mult)
        nc.vector.tensor_add(dt_[:], dt_[:], tmp1[:])
        nc.vector.tensor_scalar(dt_[:], dt_[:], 0.0, None, op0=OP.is_ge)
        nc.vector.tensor_scalar(dt_[:], dt_[:], 2.0, -1.0, op0=OP.mult, op1=OP.add)
        for i in range(3):
            nc.vector.tensor_tensor(nrm[i][:], nrm[i][:], dt_[:], OP.mult)
        # normalize
        nc.vector.tensor_tensor(tmp1[:], nrm[0][:], nrm[0][:], OP.mult)
        nc.vector.tensor_tensor(tmp2[:], nrm[1][:], nrm[1][:], OP.mult)
        nc.vector.tensor_add(tmp1[:], tmp1[:], tmp2[:])
        nc.vector.tensor_tensor(tmp2[:], nrm[2][:], nrm[2][:], OP.mult)
        nc.vector.tensor_add(tmp1[:], tmp1[:], tmp2[:])
        nc.scalar.activation(tmp1[:], tmp1[:], AF.Sqrt)
        nc.vector.tensor_scalar(tmp1[:], tmp1[:], 1e-6, None, op0=OP.max)
        nc.vector.reciprocal(tmp1[:], tmp1[:])
        for i in range(3):
            nc.vector.tensor_tensor(nrm[i][:], nrm[i][:], tmp1[:], OP.mult)

        # centered radius + dominance + plane offset
        cr = jt()
        nc.vector.tensor_tensor(cr[:], ox[:], ox[:], OP.mult)
        nc.vector.tensor_tensor(tmp1[:], oy[:], oy[:], OP.mult)
        nc.vector.tensor_add(cr[:], cr[:], tmp1[:])
        nc.vector.tensor_tensor(tmp1[:], oz[:], oz[:], OP.mult)
        nc.vector.tensor_add(cr[:], cr[:], tmp1[:])
        nc.scalar.activation(cr[:], cr[:], AF.Sqrt)
        dom = jt()
        nc.vector.tensor_scalar(tmp1[:], esum[:], 1e-6, None, op0=OP.max)
        nc.vector.reciprocal(tmp1[:], tmp1[:])
        nc.vector.tensor_tensor(dom[:], smax[:], tmp1[:], OP.mult)
        pd = jt()  # + sum(c*n); negated at assembly
        nc.vector.tensor_tensor(pd[:], cx, nrm[0][:], OP.mult)
        nc.vector.tensor_tensor(tmp1[:], cy, nrm[1][:], OP.mult)
        nc.vector.tensor_add(pd[:], pd[:], tmp1[:])
        nc.vector.tensor_tensor(tmp1[:], cz, nrm[2][:], OP.mult)
        nc.vector.tensor_add(pd[:], pd[:], tmp1[:])

        # ================= MLPs (transposed activations) =================
        inv_stage = jp.tile([128, NB, 6], f32)
        nc.vector.tensor_copy(_sl(inv_stage, 0), smin[:])
        nc.vector.tensor_copy(_sl(inv_stage, 1), smid[:])
        nc.vector.tensor_copy(_sl(inv_stage, 2), smax[:])
        nc.scalar.activation(_sl(inv_stage, 3), radii[:], AF.Copy, scale=1.0 / K)
        nc.vector.tensor_copy(_sl(inv_stage, 4), cr[:])
        nc.vector.tensor_copy(_sl(inv_stage, 5), dom[:])

        invT = jp.tile([6, N], f32)
        for rb in range(NB):
            tp = psum_small.tile([6, 128], f32, name="tp")
            nc.tensor.transpose(tp[:], inv_stage[:, rb : rb + 1, :].squeeze(1), ident[:])
            nc.scalar.activation(invT[:, rb * 128:(rb + 1) * 128], tp[:], AF.Copy)

        hidden = jp.tile([128, N], f32)
        outstage = jp.tile([128, NB, 16], f32)
        nc.vector.memset(outstage[:], 0.0)

        with (
            tc.tile_pool(name="mlp_sb", bufs=3) as mlp_sb,
            tc.tile_pool(name="ps_mlp", bufs=4, space="PSUM") as ps_mlp,
        ):
            for ch in range(8):
                cslice = slice(ch * 512, (ch + 1) * 512)
                # inv branch
                p1 = ps_mlp.tile([128, 512], f32, name="pmlp")
                nc.tensor.matmul(p1[:], w["inv_W1"][:],
                                 invT[:, cslice], start=True, stop=True)
                h1 = mlp_sb.tile([128, 512], f32)
                nc.scalar.activation(h1[:], p1[:], AF.Gelu, bias=w["inv_b1"][:])
                p2 = ps_mlp.tile([128, 512], f32, name="pmlp")
                nc.tensor.matmul(p2[:], w["inv_W2"][:],
                                 h1[:], start=True, stop=True)
                ih = mlp_sb.tile([128, 512], f32)
                nc.scalar.activation(ih[:], p2[:], AF.Identity, bias=w["inv_b2"][:])
                # feat branch
                p3 = ps_mlp.tile([128, 512], f32, name="pmlp")
                nc.tensor.matmul(p3[:], w["feat_W1"][:],
                                 featT[:, cslice], start=True, stop=True)
                h2 = mlp_sb.tile([128, 512], f32)
                nc.scalar.activation(h2[:], p3[:], AF.Gelu, bias=w["feat_b1"][:])
                p4 = ps_mlp.tile([128, 512], f32, name="pmlp")
                nc.tensor.matmul(p4[:], w["feat_W2"][:],
                                 h2[:], start=True, stop=True)
                fh = mlp_sb.tile([128, 512], f32)
                nc.scalar.activation(fh[:], p4[:], AF.Identity, bias=w["feat_b2"][:])
                # shared
                p5 = ps_mlp.tile([128, 512], f32, name="pmlp")
                nc.tensor.matmul(p5[:], w["sh_W1a"][:], ih[:],
                                 start=True, stop=False)
                nc.tensor.matmul(p5[:], w["sh_W1b"][:], fh[:],
                                 start=False, stop=True)
                hs = mlp_sb.tile([128, 512], f32)
                nc.scalar.activation(hs[:], p5[:], AF.Gelu, bias=w["sh_b1"][:])
                p6 = ps_mlp.tile([128, 512], f32, name="pmlp")
                nc.tensor.matmul(p6[:], w["sh_W2"][:], hs[:],
                                 start=True, stop=True)
                nc.scalar.activation(hidden[:, cslice], p6[:], AF.Identity,
                                     bias=w["sh_b2"][:])

            # scalar head: per row block, lhsT = hidden strip, rhs = g0_W
            for rb in range(NB):
                sp = ps_mlp.tile([128, 1], f32, name="psp", bufs=2)
                nc.tensor.matmul(sp[:], hidden[:, rb * 128:(rb + 1) * 128], w["g0_W"][:],
                                 start=True, stop=True)
                nc.scalar.activation(outstage[:, rb : rb + 1, 0:1].squeeze(1), sp[:], AF.Identity,
                                     bias=w["g0_b"][:])

        # ================= assembly + output DMA =================
        nc.vector.tensor_copy(_sl(outstage, 1), nrm[0][:])
        nc.vector.tensor_copy(_sl(outstage, 2), nrm[1][:])
        nc.scalar.activation(_sl(outstage, 3), nrm[2][:], AF.Copy, scale=-1.0)
        nc.scalar.activation(_sl(outstage, 4), pd[:], AF.Copy, scale=-1.0)
        nc.vector.tensor_copy(_sl(outstage, 5), nrm[0][:])
        nc.vector.tensor_copy(_sl(outstage, 6), nrm[1][:])
        nc.vector.tensor_copy(_sl(outstage, 7), nrm[2][:])
        nc.vector.tensor_copy(_sl(outstage, 11), cx)
        nc.vector.tensor_copy(_sl(outstage, 12), cy)
        nc.vector.tensor_copy(_sl(outstage, 13), cz)
        nc.vector.memset(_sl(outstage, 14), 1.0)

        for rb in range(NB):
            nc.sync.dma_start(out_h[rb * 128:(rb + 1) * 128, :], outstage[:, rb : rb + 1, :].squeeze(1))


# ---------------------------------------------------------------------------
# SPMD driver: batch b -> core b (persistent PJRT executable, axon-tunneled)
# ---------------------------------------------------------------------------
_WKEYS = ("inv_W1", "inv_b1", "inv_W2", "inv_b2", "feat_W1", "feat_b1",
          "feat_W2", "feat_b2", "sh_W1", "sh_b1", "sh_W2", "sh_b2",
          "g0_W", "g0_b")
_B = 8
_CACHE = {}


def _get_runner():
    if "runner" in _CACHE:
        return _CACHE["runner"]
    import jax
    from jax.sharding import Mesh, PartitionSpec
    from jax.experimental.shard_map import shard_map
    from concourse import bass2jax, mybir as _mb
    from concourse.bass2jax import _bass_exec_p, install_neuronx_cc_hook

    install_neuronx_cc_hook()
    nc = build_nc()
    partition_name = nc.partition_id_tensor.name if nc.partition_id_tensor else None
    in_names, out_names, out_avals = [], [], []
    for alloc in nc.m.functions[0].allocations:
        if not isinstance(alloc, _mb.MemoryLocationSet):
            continue
        name = alloc.memorylocations[0].name
        if alloc.kind == "ExternalInput":
            if name != partition_name:
                in_names.append(name)
        elif alloc.kind == "ExternalOutput":
            out_avals.append(jax.core.ShapedArray(tuple(alloc.tensor_shape),
                                                  _mb.dt.np(alloc.dtype)))
            out_names.append(name)
    n_params = len(in_names)
    n_outs = len(out_avals)
    all_in = list(in_names) + list(out_names)
    if partition_name is not None:
        all_in.append(partition_name)

    def _body(*args):
        operands = list(args)
        if partition_name is not None:
            operands.append(bass2jax.partition_id_tensor())
        return tuple(_bass_exec_p.bind(
            *operands, out_avals=tuple(out_avals), in_names=tuple(all_in),
            out_names=tuple(out_names), lowering_input_output_aliases=(),
            sim_require_finite=True, sim_require_nnan=True, nc=nc))

    devices = jax.devices()[:_B]
    mesh = Mesh(np.asarray(devices), ("core",))
    sharded = jax.jit(
        shard_map(_body, mesh=mesh,
                  in_specs=(PartitionSpec("core"),) * (n_params + n_outs),
                  out_specs=(PartitionSpec("core"),) * n_outs,
                  check_rep=False),
        keep_unused=True)
    zeros_dev = [jax.device_put(
        np.zeros((_B * a.shape[0],) + tuple(a.shape[1:]), a.dtype))
        for a in out_avals]
    _CACHE["runner"] = (sharded, in_names, out_names, out_avals, zeros_dev)
    return _CACHE["runner"]


def _concat_inputs(inputs, in_names):
    per_core = []
    for b in range(_B):
        m = {
            "coords": np.ascontiguousarray(inputs["coords"][b], dtype=np.float32),
            "features": np.ascontiguousarray(inputs["features"][b], dtype=np.float32),
        }
        for k in _WKEYS:
            m[k] = np.ascontiguousarray(np.asarray(inputs[k], dtype=np.float32))
        per_core.append([m[n] for n in in_names])
    return [np.concatenate([per_core[c][i] for c in range(_B)], axis=0)
            for i in range(len(in_names))]


def run(inputs):
    sharded, in_names, out_names, out_avals, zeros_dev = _get_runner()
    outs = sharded(*_concat_inputs(inputs, in_names), *zeros_dev)
    i = out_names.index("out")
    return np.asarray(outs[i]).reshape(_B, *out_avals[i].shape).astype(np.float32)


def time_kernel(inputs, iters=20):
    """Steady-state per-invocation wall time (ns), pipelined dispatch."""
    import time as _time
    import jax
    sharded, in_names, out_names, out_avals, zeros_dev = _get_runner()
    dev_in = [jax.device_put(a) for a in _concat_inputs(inputs, in_names)]
    for _ in range(2):  # warm
        jax.block_until_ready(sharded(*dev_in, *zeros_dev))
    t0 = _time.perf_counter()
    outs = [sharded(*dev_in, *zeros_dev) for _ in range(iters)]
    jax.block_until_ready(outs)
    return (_time.perf_counter() - t0) / iters * 1e9


def kernel(**inputs):
    return run(inputs)


# revision 7
# speedup vs baseline: 38.7156x; 2.8590x over previous
"""Bass/Tile kernel for nn_PointCloudMPE: per-core pipeline for one batch
(coords [4096,3], features [4096,64]) -> out [4096,16].

Pipeline: pairwise -dist^2 via PE matmul (homogeneous 5-dim trick) ->
top-16 threshold via DVE max8/match_replace/max8 -> 0/1 neighbor mask ->
masked moment sums via PE matmuls (mask transposed on PE) -> 3x3 covariance
-> batched branch-free cyclic Jacobi eigensolver -> normals/invariants ->
MLPs on PE (transposed activations) -> assembled [4096,16] output.
"""

import numpy as np

import concourse.bass as bass
import concourse.bacc as bacc
import concourse.mybir as mybir
import concourse.tile as tile
from concourse.masks import make_identity

f32 = mybir.dt.float32
f32r = mybir.dt.float32r
AF = mybir.ActivationFunctionType
OP = mybir.AluOpType

N, K, NB = 4096, 16, 32  # points, knn, row blocks of 128
NEG_BIG = -1.0e30

# packed input layout (flat f32 offsets)
WSHAPES = [("coords", (N, 3)), ("features", (N, 64)),
           ("inv_W1", (6, 128)), ("inv_b1", (128,)), ("inv_W2", (128, 128)),
           ("inv_b2", (128,)), ("feat_W1", (64, 128)), ("feat_b1", (128,)),
           ("feat_W2", (128, 128)), ("feat_b2", (128,)), ("sh_W1", (256, 128)),
           ("sh_b1", (128,)), ("sh_W2", (128, 128)), ("sh_b2", (128,)),
           ("g0_W", (128, 1)), ("g0_b", (1,))]
POFS = {}
_o = 0
for _n, _s in WSHAPES:
    POFS[_n] = _o
    _o += int(np.prod(_s))
PACKED_LEN = _o


def _sl(t, q):
    # [128, 32, C] tile -> [128, 32] strided view of component q
    return t[:, :, q : q + 1].squeeze(2)


def build_nc():
    nc = bacc.Bacc("TRN2", target_bir_lowering=False)

    packed_h = nc.declare_dram_parameter("packed", [PACKED_LEN], f32, isOutput=False)
    out_h = nc.declare_dram_parameter("out", [N, 16], f32, isOutput=True)

    with tile.TileContext(nc) as tc:
        _body(nc, tc, packed_h, out_h)
    nc.finalize()
    return nc


def _pk(packed_h, name, shape):
    ofs = POFS[name]
    n = int(np.prod(shape))
    ap = packed_h[ofs:ofs + n]
    if len(shape) == 1:
        return ap.rearrange("(a b) -> a b", b=1)
    assert len(shape) == 2
    return ap.rearrange("(a b) -> a b", b=shape[1])


def _body(nc, tc, packed_h, out_h):
    import contextlib

    ctx = contextlib.ExitStack()
    with ctx:
        persist = ctx.enter_context(tc.tile_pool(name="persist", bufs=1))
        psum_small = ctx.enter_context(tc.tile_pool(name="ps_small", bufs=2, space="PSUM"))

        ident = persist.tile([128, 128], f32)
        make_identity(nc, ident[:])

        # ---- load coords in block layout + weights ----
        coords_nat = persist.tile([128, NB, 3], f32)
        nc.sync.dma_start(
            coords_nat[:],
            packed_h[POFS["coords"]:POFS["coords"] + N * 3]
            .rearrange("(rb p d) -> p rb d", p=128, d=3))

        wdims = {"inv_W1": (6, 128), "inv_W2": (128, 128), "feat_W1": (64, 128),
                 "feat_W2": (128, 128), "sh_W2": (128, 128), "g0_W": (128, 1)}
        w = {}
        for k, shp in wdims.items():
            w[k] = persist.tile(list(shp), f32, name=f"w_{k}")
            nc.sync.dma_start(w[k][:], _pk(packed_h, k, shp))
        w["sh_W1a"] = persist.tile([128, 128], f32, name="w_sh_W1a")
        w["sh_W1b"] = persist.tile([128, 128], f32, name="w_sh_W1b")
        sh1 = _pk(packed_h, "sh_W1", (256, 128))
        nc.sync.dma_start(w["sh_W1a"][:], sh1[0:128, :])
        nc.sync.dma_start(w["sh_W1b"][:], sh1[128:256, :])
        for k in ("inv_b1", "inv_b2", "feat_b1", "feat_b2", "sh_b1", "sh_b2"):
            w[k] = persist.tile([128, 1], f32, name=f"w_{k}")
            nc.sync.dma_start(w[k][:], _pk(packed_h, k, (128,)))
        g0b_one = persist.tile([1, 1], f32)
        nc.sync.dma_start(g0b_one[:], _pk(packed_h, "g0_b", (1,)))
        ones_row = persist.tile([1, 128], f32)
        nc.vector.memset(ones_row[:], 1.0)
        w["g0_b"] = persist.tile([128, 1], f32, name="w_g0_b_bc")
        g0ps = psum_small.tile([128, 1], f32, name="tp")
        nc.tensor.matmul(g0ps[:], ones_row[:], g0b_one[:], start=True, stop=True)
        nc.scalar.activation(w["g0_b"][:], g0ps[:], AF.Copy)

        # ---- phi [128, 32, 9]: x,y,z,xx,xy,xz,yy,yz,zz per point ----
        phi = persist.tile([128, NB, 9], f32)
        nc.vector.tensor_copy(phi[:, :, 0:3], coords_nat[:])
        pq = [(0, 0), (0, 1), (0, 2), (1, 1), (1, 2), (2, 2)]
        for qi, (a, b) in enumerate(pq):
            nc.vector.tensor_tensor(
                phi[:, :, 3 + qi : 4 + qi], coords_nat[:, :, a : a + 1],
                coords_nat[:, :, b : b + 1], OP.mult)

        # ---- L/R matrices for -dist^2 matmul, via block-layout staging ----
        # L col j: (x,y,z,sq,1); R col j: (2x,2y,2z,-1,-sq)
        sq_nat = persist.tile([128, NB], f32)
        nc.vector.tensor_add(sq_nat[:], _sl(phi, 3), _sl(phi, 6))
        nc.vector.tensor_add(sq_nat[:], sq_nat[:], _sl(phi, 8))
        Lstage = persist.tile([128, NB, 5], f32)
        Rstage = persist.tile([128, NB, 5], f32)
        nc.vector.tensor_copy(Lstage[:, :, 0:3], coords_nat[:])
        nc.vector.tensor_copy(_sl(Lstage, 3), sq_nat[:])
        nc.vector.memset(_sl(Lstage, 4), 1.0)
        nc.scalar.activation(Rstage[:, :, 0:3], coords_nat[:], AF.Copy, scale=2.0)
        nc.vector.memset(_sl(Rstage, 3), -1.0)
        nc.scalar.activation(_sl(Rstage, 4), sq_nat[:], AF.Copy, scale=-1.0)

        L_all = persist.tile([5, N], f32)
        R_all = persist.tile([5, N], f32)
        for rb in range(NB):
            tpl = psum_small.tile([5, 128], f32, name="tp")
            nc.tensor.transpose(tpl[:], Lstage[:, rb : rb + 1, :].squeeze(1), ident[:])
            nc.scalar.activation(L_all[:, rb * 128:(rb + 1) * 128], tpl[:], AF.Copy)
            tpr = psum_small.tile([5, 128], f32, name="tp")
            nc.tensor.transpose(tpr[:], Rstage[:, rb : rb + 1, :].squeeze(1), ident[:])
            nc.scalar.activation(R_all[:, rb * 128:(rb + 1) * 128], tpr[:], AF.Copy)

        # ---- featT [64, 4096] ----
        featT = persist.tile([64, N], f32)
        with tc.tile_pool(name="ftile", bufs=2) as fpool:
            for rb in range(NB):
                ft = fpool.tile([128, 64], f32)
                nc.sync.dma_start(
                    ft[:],
                    packed_h[POFS["features"] + rb * 128 * 64:
                             POFS["features"] + (rb + 1) * 128 * 64]
                    .rearrange("(p d) -> p d", d=64))
                tp = psum_small.tile([64, 128], f32, name="tp")
                nc.tensor.transpose(tp[:], ft[:], ident[:])
                nc.scalar.activation(featT[:, rb * 128:(rb + 1) * 128], tp[:], AF.Copy)

        # ---- per-point accumulators (block layout [128, 32]) ----
        radii = persist.tile([128, NB], f32)
        Sall = persist.tile([128, 9, NB], f32)

        # ---- main selection + moment loop over row blocks ----
        with (
            tc.tile_pool(name="ndpool", bufs=2) as ndpool,
            tc.tile_pool(name="scrpool", bufs=2) as scrpool,
            tc.tile_pool(name="mpool", bufs=2) as mpool,
            tc.tile_pool(name="mtpool", bufs=2) as mtpool,
            tc.tile_pool(name="selpool", bufs=2) as selpool,
            tc.tile_pool(name="ps_nd", bufs=2, space="PSUM") as ps_nd,
            tc.tile_pool(name="ps_mt", bufs=2, space="PSUM") as ps_mt,
            tc.tile_pool(name="ps_s", bufs=2, space="PSUM") as ps_s,
        ):
            def compute_nd(rb):
                nd = ndpool.tile([128, N], f32, name="nd")
                lhs = L_all[:, rb * 128:(rb + 1) * 128]
                for ct in range(8):
                    ps = ps_nd.tile([128, 512], f32, name="ps")
                    nc.tensor.matmul(ps[:], lhs, R_all[:, ct * 512:(ct + 1) * 512],
                                     start=True, stop=True)
                    nc.scalar.activation(nd[:, ct * 512:(ct + 1) * 512], ps[:], AF.Copy)
                # self-exclusion: diagonal block columns rb*128..rb*128+127
                nc.gpsimd.affine_select(
                    out=nd[:, rb * 128:(rb + 1) * 128],
                    in_=nd[:, rb * 128:(rb + 1) * 128],
                    compare_op=OP.not_equal, fill=NEG_BIG,
                    base=0, pattern=[[1, 128]], channel_multiplier=-1)
                return nd

            nd_next = compute_nd(0)
            for rb in range(NB):
                nd = nd_next
                if rb + 1 < NB:
                    nd_next = compute_nd(rb + 1)

                mx1 = selpool.tile([128, 8], f32)
                mx2 = selpool.tile([128, 8], f32)
                scr = scrpool.tile([128, N], f32)
                nc.vector.max(out=mx1[:], in_=nd[:])
                nc.vector.match_replace(out=scr[:], in_to_replace=mx1[:],
                                        in_values=nd[:], imm_value=NEG_BIG)
                nc.vector.max(out=mx2[:], in_=scr[:])

                mask = mpool.tile([128, N], f32)
                nc.gpsimd.tensor_scalar(mask[:], nd[:], mx2[:, 7:8], None, op0=OP.is_ge)

                # radius: the 16 selected -dist^2 values are mx1/mx2 [128,8];
                # sum sqrt(dist) via two tiny clamped sqrts with accumulate
                mc1 = selpool.tile([128, 8], f32, name="mc1")
                mc2 = selpool.tile([128, 8], f32, name="mc2")
                nc.vector.tensor_scalar(mc1[:], mx1[:], 0.0, None, op0=OP.min)
                nc.vector.tensor_scalar(mc2[:], mx2[:], 0.0, None, op0=OP.min)
                r1 = selpool.tile([128, 1], f32, name="r1")
                r2 = selpool.tile([128, 1], f32, name="r2")
                nc.scalar.activation(mc1[:], mc1[:], AF.Sqrt, scale=-1.0,
                                     accum_out=r1[:])
                nc.scalar.activation(mc2[:], mc2[:], AF.Sqrt, scale=-1.0,
                                     accum_out=r2[:])
                nc.vector.tensor_add(radii[:, rb : rb + 1], r1[:], r2[:])

                # transpose mask strip -> MT [j, i] for this row block
                mt = mtpool.tile([128, N], f32)
                for g in range(8):
                    tp = ps_mt.tile([128, 512], f32)
                    for kk in range(4):
                        c0 = (4 * g + kk) * 128
                        nc.tensor.transpose(tp[:, kk * 128:(kk + 1) * 128],
                                            mask[:, c0:c0 + 128], ident[:])
                    nc.scalar.activation(mt[:, g * 512:(g + 1) * 512], tp[:], AF.Copy)

                sps = ps_s.tile([128, 9], f32)
                for jb in range(NB):
                    nc.tensor.matmul(sps[:], mt[:, jb * 128:(jb + 1) * 128],
                                     phi[:, jb : jb + 1, :].squeeze(1), start=(jb == 0), stop=(jb == NB - 1))
                nc.scalar.activation(Sall[:, :, rb : rb + 1].squeeze(2), sps[:], AF.Copy)

        # ================= covariance + Jacobi + invariants =================
        jp = ctx.enter_context(tc.tile_pool(name="jacobi", bufs=1))

        _jtc = [0]

        def jt():
            _jtc[0] += 1
            return jp.tile([128, NB], f32, name=f"jt{_jtc[0]}")

        cx, cy, cz = (_sl(coords_nat, q) for q in range(3))
        inv_k = 1.0 / K

        # neighbor means
        m1 = [jp.tile([128, NB], f32, name=f"m1_{q}") for q in range(3)]
        for q in range(3):
            nc.scalar.activation(m1[q][:], Sall[:, q : q + 1, :].squeeze(1), AF.Copy, scale=inv_k)

        # cov components: S2ab/K - ca*m1b - cb*m1a + ca*cb
        cvs = [cx, cy, cz]
        A = {}
        for qi, (a, b) in enumerate(pq):
            t0 = jt()
            nc.scalar.activation(t0[:], Sall[:, 3 + qi : 4 + qi, :].squeeze(1), AF.Copy, scale=inv_k)
            t1 = jt()
            nc.vector.tensor_tensor(t1[:], cvs[a], m1[b][:], OP.mult)
            nc.vector.tensor_sub(t0[:], t0[:], t1[:])
            nc.vector.tensor_tensor(t1[:], cvs[b], m1[a][:], OP.mult)
            nc.vector.tensor_sub(t0[:], t0[:], t1[:])
            nc.vector.tensor_tensor(t1[:], cvs[a], cvs[b], OP.mult)
            nc.vector.tensor_add(t0[:], t0[:], t1[:])
            A[(a, b)] = t0

        # V = I (9 tiles)
        V = {}
        for i in range(3):
            for j in range(3):
                V[(i, j)] = jp.tile([128, NB], f32, name=f"V{i}{j}")
                nc.vector.memset(V[(i, j)][:], 1.0 if i == j else 0.0)

        def Ag(i, j):
            return A[(i, j)] if i <= j else A[(j, i)]

        tmp1, tmp2, tmp3, tmp4 = jt(), jt(), jt(), jt()
        SWEEPS = 4
        for sw in range(SWEEPS):
            for (p, q) in ((0, 1), (0, 2), (1, 2)):
                app, aqq, apq = Ag(p, p), Ag(q, q), Ag(p, q)
                # theta = (aqq - app) / (2*apq), guarded; t = sgn/( |th| + sqrt(th^2+1) )
                sA = tmp1
                nc.vector.tensor_scalar(sA[:], apq[:], 0.0, None, op0=OP.is_ge)
                nc.vector.tensor_scalar(sA[:], sA[:], 2.0, -1.0, op0=OP.mult, op1=OP.add)
                den = tmp2
                nc.vector.tensor_tensor(den[:], apq[:], sA[:], OP.mult)  # |apq|
                nc.vector.tensor_scalar(den[:], den[:], 1e-30, 2.0, op0=OP.max, op1=OP.mult)
                nc.vector.reciprocal(den[:], den[:])                     # 1/(2|apq|)
                th = tmp3
                nc.vector.tensor_sub(th[:], app[:], aqq[:])
                nc.vector.tensor_tensor(th[:], th[:], den[:], OP.mult)
                nc.vector.tensor_tensor(th[:], th[:], sA[:], OP.mult)    # theta (signed)
                nc.vector.tensor_scalar(th[:], th[:], 1e17, -1e17, op0=OP.min, op1=OP.max)
                sT = tmp1
                nc.vector.tensor_scalar(sT[:], th[:], 0.0, None, op0=OP.is_ge)
                nc.vector.tensor_scalar(sT[:], sT[:], 2.0, -1.0, op0=OP.mult, op1=OP.add)
                tha = tmp2
                nc.vector.tensor_tensor(tha[:], th[:], sT[:], OP.mult)   # |theta|
                r_ = tmp4
                nc.vector.tensor_tensor(r_[:], th[:], th[:], OP.mult)
                nc.scalar.activation(r_[:], r_[:], AF.Sqrt, bias=1.0)    # sqrt(th^2+1)
                nc.vector.tensor_add(r_[:], r_[:], tha[:])
                nc.vector.reciprocal(r_[:], r_[:])
                tt = tmp2
                nc.vector.tensor_tensor(tt[:], r_[:], sT[:], OP.mult)    # tan(phi)
                cc_ = tmp3
                nc.vector.tensor_tensor(cc_[:], tt[:], tt[:], OP.mult)
                nc.scalar.activation(cc_[:], cc_[:], AF.Sqrt, bias=1.0)
                nc.vector.reciprocal(cc_[:], cc_[:])                     # cos(phi)
                ss_ = tmp4
                nc.vector.tensor_tensor(ss_[:], tt[:], cc_[:], OP.mult)  # sin(phi)
                cs, sn = cc_, ss_

                # update A
                c2, s2, csn = jt(), jt(), jt()
                nc.vector.tensor_tensor(c2[:], cs[:], cs[:], OP.mult)
                nc.vector.tensor_tensor(s2[:], sn[:], sn[:], OP.mult)
                nc.vector.tensor_tensor(csn[:], cs[:], sn[:], OP.mult)
                u1, u2, u3, u4 = jt(), jt(), jt(), jt()
                nc.vector.tensor_tensor(u1[:], c2[:], app[:], OP.mult)
                nc.vector.tensor_tensor(u2[:], csn[:], apq[:], OP.mult)
                nc.vector.tensor_tensor(u3[:], s2[:], aqq[:], OP.mult)
                nc.vector.tensor_tensor(u4[:], s2[:], app[:], OP.mult)
                nc.gpsimd.tensor_tensor(app[:], c2[:], aqq[:], OP.mult)  # app <- c2*aqq (tmp)
                # app' = u1 + 2*u2 + u3 ; aqq' = u4 - 2*u2 + c2*aqq
                nc.vector.scalar_tensor_tensor(u1[:], u2[:], 2.0, u1[:], op0=OP.mult, op1=OP.add)
                nc.vector.tensor_add(u1[:], u1[:], u3[:])
                nc.vector.scalar_tensor_tensor(u4[:], u2[:], -2.0, u4[:], op0=OP.mult, op1=OP.add)
                nc.vector.tensor_add(aqq[:], u4[:], app[:])
                nc.vector.tensor_copy(app[:], u1[:])
                nc.vector.memset(apq[:], 0.0)
                rr = 3 - p - q
                apr, aqr = Ag(p, rr), Ag(q, rr)
                nc.vector.tensor_tensor(u1[:], cs[:], apr[:], OP.mult)
                nc.vector.tensor_tensor(u2[:], sn[:], aqr[:], OP.mult)
                nc.vector.tensor_tensor(u3[:], sn[:], apr[:], OP.mult)
                nc.vector.tensor_tensor(u4[:], cs[:], aqr[:], OP.mult)
                nc.vector.tensor_add(apr[:], u1[:], u2[:])
                nc.vector.tensor_sub(aqr[:], u4[:], u3[:])
                # update V columns p,q (rows 0..2) on gpsimd
                for i in range(3):
                    vip, viq = V[(i, p)], V[(i, q)]
                    nc.gpsimd.tensor_tensor(u1[:], cs[:], vip[:], OP.mult)
                    nc.gpsimd.tensor_tensor(u2[:], sn[:], viq[:], OP.mult)
                    nc.gpsimd.tensor_tensor(u3[:], sn[:], vip[:], OP.mult)
                    nc.gpsimd.tensor_tensor(u4[:], cs[:], viq[:], OP.mult)
                    nc.gpsimd.tensor_add(vip[:], u1[:], u2[:])
                    nc.gpsimd.tensor_sub(viq[:], u4[:], u3[:])

        e0, e1, e2 = A[(0, 0)], A[(1, 1)], A[(2, 2)]
        smin, smid, smax = jt(), jt(), jt()
        nc.vector.tensor_tensor(smin[:], e0[:], e1[:], OP.min)
        nc.vector.tensor_tensor(smin[:], smin[:], e2[:], OP.min)
        nc.vector.tensor_tensor(smax[:], e0[:], e1[:], OP.max)
        nc.vector.tensor_tensor(smax[:], smax[:], e2[:], OP.max)
        nc.vector.tensor_add(smid[:], e0[:], e1[:])
        nc.vector.tensor_add(smid[:], smid[:], e2[:])
        esum = jt()
        nc.vector.tensor_copy(esum[:], smid[:])  # e0+e1+e2
        nc.vector.tensor_sub(smid[:], smid[:], smin[:])
        nc.vector.tensor_sub(smid[:], smid[:], smax[:])

        # select eigenvector of smallest eigenvalue: exclusive masks
        m0, m1m, m2m = jt(), jt(), jt()
        nc.vector.tensor_tensor(tmp1[:], e0[:], e1[:], OP.is_le)
        nc.vector.tensor_tensor(tmp2[:], e0[:], e2[:], OP.is_le)
        nc.vector.tensor_tensor(m0[:], tmp1[:], tmp2[:], OP.mult)
        nc.vector.tensor_tensor(tmp1[:], e1[:], e0[:], OP.is_lt)
        nc.vector.tensor_tensor(tmp2[:], e1[:], e2[:], OP.is_le)
        nc.vector.tensor_tensor(m1m[:], tmp1[:], tmp2[:], OP.mult)
        nc.vector.tensor_add(m2m[:], m0[:], m1m[:])
        nc.vector.tensor_scalar(m2m[:], m2m[:], -1.0, 1.0, op0=OP.mult, op1=OP.add)

        nrm = [jp.tile([128, NB], f32, name=f"nrm{q}") for q in range(3)]  # normals nx,ny,nz
        for i in range(3):
            nc.vector.tensor_tensor(tmp1[:], m0[:], V[(i, 0)][:], OP.mult)
            nc.vector.tensor_tensor(tmp2[:], m1m[:], V[(i, 1)][:], OP.mult)
            nc.vector.tensor_add(tmp1[:], tmp1[:], tmp2[:])
            nc.vector.tensor_tensor(tmp2[:], m2m[:], V[(i, 2)][:], OP.mult)
            nc.vector.tensor_add(nrm[i][:], tmp1[:], tmp2[:])

        # center + outward + orientation
        ones128 = jp.tile([128, 1], f32)
        nc.vector.memset(ones128[:], 1.0)
        partials = jp.tile([128, 3], f32)
        for q, cv in enumerate(cvs):
            nc.vector.tensor_reduce(partials[:, q : q + 1], cv, axis=mybir.AxisListType.X,
                                    op=OP.add)
        cps = psum_small.tile([1, 3], f32, name="tp")
        nc.tensor.matmul(cps[:], ones128[:], partials[:], start=True, stop=True)
        center1 = jp.tile([1, 3], f32)
        nc.scalar.activation(center1[:], cps[:], AF.Copy, scale=1.0 / N)
        center = jp.tile([128, 3], f32)
        cbps = psum_small.tile([128, 3], f32, name="tp")
        nc.tensor.matmul(cbps[:], ones_row[:], center1[:], start=True, stop=True)
        nc.scalar.activation(center[:], cbps[:], AF.Copy)

        ox, oy, oz = jt(), jt(), jt()
        ovs = [ox, oy, oz]
        for q in range(3):
            nc.vector.tensor_scalar(ovs[q][:], cvs[q], center[:, q : q + 1], None,
                                    op0=OP.subtract)
        dt_ = jt()
        nc.vector.tensor_tensor(dt_[:], nrm[0][:], ox[:], OP.mult)
        nc.vector.tensor_tensor(tmp1[:], nrm[1][:], oy[:], OP.mult)
        nc.vector.tensor_add(dt_[:], dt_[:], tmp1[:])
        nc.vector.tensor_tensor(tmp1[:], nrm[2][:], oz[:], OP.mult)
        nc.vector.tensor_add(dt_[:], dt_[:], tmp1[:])
        nc.vector.tensor_scalar(dt_[:], dt_[:], 0.0, None, op0=OP.is_ge)
        nc.vector.tensor_scalar(dt_[:], dt_[:], 2.0, -1.0, op0=OP.mult, op1=OP.add)
        for i in range(3):
            nc.vector.tensor_tensor(nrm[i][:], nrm[i][:], dt_[:], OP.mult)
        # normalize
        nc.vector.tensor_tensor(tmp1[:], nrm[0][:], nrm[0][:], OP.mult)
        nc.vector.tensor_tensor(tmp2[:], nrm[1][:], nrm[1][:], OP.mult)
        nc.vector.tensor_add(tmp1[:], tmp1[:], tmp2[:])
        nc.vector.tensor_tensor(tmp2[:], nrm[2][:], nrm[2][:], OP.mult)
        nc.vector.tensor_add(tmp1[:], tmp1[:], tmp2[:])
        nc.scalar.activation(tmp1[:], tmp1[:], AF.Sqrt)
        nc.vector.tensor_scalar(tmp1[:], tmp1[:], 1e-6, None, op0=OP.max)
        nc.vector.reciprocal(tmp1[:], tmp1[:])
        for i in range(3):
            nc.vector.tensor_tensor(nrm[i][:], nrm[i][:], tmp1[:], OP.mult)

        # centered radius + dominance + plane offset
        cr = jt()
        nc.vector.tensor_tensor(cr[:], ox[:], ox[:], OP.mult)
        nc.vector.tensor_tensor(tmp1[:], oy[:], oy[:], OP.mult)
        nc.vector.tensor_add(cr[:], cr[:], tmp1[:])
        nc.vector.tensor_tensor(tmp1[:], oz[:], oz[:], OP.mult)
        nc.vector.tensor_add(cr[:], cr[:], tmp1[:])
        nc.scalar.activation(cr[:], cr[:], AF.Sqrt)
        dom = jt()
        nc.vector.tensor_scalar(tmp1[:], esum[:], 1e-6, None, op0=OP.max)
        nc.vector.reciprocal(tmp1[:], tmp1[:])
        nc.vector.tensor_tensor(dom[:], smax[:], tmp1[:], OP.mult)
        pd = jt()  # + sum(c*n); negated at assembly
        nc.vector.tensor_tensor(pd[:], cx, nrm[0][:], OP.mult)
        nc.vector.tensor_tensor(tmp1[:], cy, nrm[1][:], OP.mult)
        nc.vector.tensor_add(pd[:], pd[:], tmp1[:])
        nc.vector.tensor_tensor(tmp1[:], cz, nrm[2][:], OP.mult)
        nc.vector.tensor_add(pd[:], pd[:], tmp1[:])

        # ================= MLPs (transposed activations) =================
        inv_stage = jp.tile([128, NB, 6], f32)
        nc.vector.tensor_copy(_sl(inv_stage, 0), smin[:])
        nc.vector.tensor_copy(_sl(inv_stage, 1), smid[:])
        nc.vector.tensor_copy(_sl(inv_stage, 2), smax[:])
        nc.scalar.activation(_sl(inv_stage, 3), radii[:], AF.Copy, scale=1.0 / K)
        nc.vector.tensor_copy(_sl(inv_stage, 4), cr[:])
        nc.vector.tensor_copy(_sl(inv_stage, 5), dom[:])

        invT = jp.tile([6, N], f32)
        for rb in range(NB):
            tp = psum_small.tile([6, 128], f32, name="tp")
            nc.tensor.transpose(tp[:], inv_stage[:, rb : rb + 1, :].squeeze(1), ident[:])
            nc.scalar.activation(invT[:, rb * 128:(rb + 1) * 128], tp[:], AF.Copy)

        hidden = jp.tile([128, N], f32)
        outstage = jp.tile([128, NB, 16], f32)
        nc.vector.memset(outstage[:], 0.0)

        with (
            tc.tile_pool(name="mlp_sb", bufs=3) as mlp_sb,
            tc.tile_pool(name="ps_mlp", bufs=4, space="PSUM") as ps_mlp,
        ):
            for ch in range(8):
                cslice = slice(ch * 512, (ch + 1) * 512)
                # inv branch
                p1 = ps_mlp.tile([128, 512], f32, name="pmlp")
                nc.tensor.matmul(p1[:], w["inv_W1"][:],
                                 invT[:, cslice], start=True, stop=True)
                h1 = mlp_sb.tile([128, 512], f32)
                nc.scalar.activation(h1[:], p1[:], AF.Gelu, bias=w["inv_b1"][:])
                p2 = ps_mlp.tile([128, 512], f32, name="pmlp")
                nc.tensor.matmul(p2[:], w["inv_W2"][:],
                                 h1[:], start=True, stop=True)
                ih = mlp_sb.tile([128, 512], f32)
                nc.scalar.activation(ih[:], p2[:], AF.Identity, bias=w["inv_b2"][:])
                # feat branch
                p3 = ps_mlp.tile([128, 512], f32, name="pmlp")
                nc.tensor.matmul(p3[:], w["feat_W1"][:],
                                 featT[:, cslice], start=True, stop=True)
                h2 = mlp_sb.tile([128, 512], f32)
                nc.scalar.activation(h2[:], p3[:], AF.Gelu, bias=w["feat_b1"][:])
                p4 = ps_mlp.tile([128, 512], f32, name="pmlp")
                nc.tensor.matmul(p4[:], w["feat_W2"][:],
                                 h2[:], start=True, stop=True)
                fh = mlp_sb.tile([128, 512], f32)
                nc.scalar.activation(fh[:], p4[:], AF.Identity, bias=w["feat_b2"][:])
                # shared
                p5 = ps_mlp.tile([128, 512], f32, name="pmlp")
                nc.tensor.matmul(p5[:], w["sh_W1a"][:], ih[:],
                                 start=True, stop=False)
                nc.tensor.matmul(p5[:], w["sh_W1b"][:], fh[:],
                                 start=False, stop=True)
                hs = mlp_sb.tile([128, 512], f32)
                nc.scalar.activation(hs[:], p5[:], AF.Gelu, bias=w["sh_b1"][:])
                p6 = ps_mlp.tile([128, 512], f32, name="pmlp")
                nc.tensor.matmul(p6[:], w["sh_W2"][:], hs[:],
                                 start=True, stop=True)
                nc.scalar.activation(hidden[:, cslice], p6[:], AF.Identity,
                                     bias=w["sh_b2"][:])

            # scalar head: per row block, lhsT = hidden strip, rhs = g0_W
            for rb in range(NB):
                sp = ps_mlp.tile([128, 1], f32, name="psp", bufs=2)
                nc.tensor.matmul(sp[:], hidden[:, rb * 128:(rb + 1) * 128], w["g0_W"][:],
                                 start=True, stop=True)
                nc.scalar.activation(outstage[:, rb : rb + 1, 0:1].squeeze(1), sp[:], AF.Identity,
                                     bias=w["g0_b"][:])

        # ================= assembly + output DMA =================
        nc.vector.tensor_copy(_sl(outstage, 1), nrm[0][:])
        nc.vector.tensor_copy(_sl(outstage, 2), nrm[1][:])
        nc.scalar.activation(_sl(outstage, 3), nrm[2][:], AF.Copy, scale=-1.0)
        nc.scalar.activation(_sl(outstage, 4), pd[:], AF.Copy, scale=-1.0)
        nc.vector.tensor_copy(_sl(outstage, 5), nrm[0][:])
        nc.vector.tensor_copy(_sl(outstage, 6), nrm[1][:])
        nc.vector.tensor_copy(_sl(outstage, 7), nrm[2][:])
        nc.vector.tensor_copy(_sl(outstage, 11), cx)
        nc.vector.tensor_copy(_sl(outstage, 12), cy)
        nc.vector.tensor_copy(_sl(outstage, 13), cz)
        nc.vector.memset(_sl(outstage, 14), 1.0)

        for rb in range(NB):
            nc.sync.dma_start(out_h[rb * 128:(rb + 1) * 128, :], outstage[:, rb : rb + 1, :].squeeze(1))


# ---------------------------------------------------------------------------
# SPMD driver: batch b -> core b (persistent PJRT executable, packed input)
# ---------------------------------------------------------------------------
_B = 8
_CACHE = {}


def _get_runner():
    if "runner" in _CACHE:
        return _CACHE["runner"]
    import jax
    from jax.sharding import Mesh, PartitionSpec
    from jax.experimental.shard_map import shard_map
    from concourse import bass2jax, mybir as _mb
    from concourse.bass2jax import _bass_exec_p, install_neuronx_cc_hook

    install_neuronx_cc_hook()
    nc = build_nc()
    partition_name = nc.partition_id_tensor.name if nc.partition_id_tensor else None
    in_names, out_names, out_avals = [], [], []
    for alloc in nc.m.functions[0].allocations:
        if not isinstance(alloc, _mb.MemoryLocationSet):
            continue
        name = alloc.memorylocations[0].name
        if alloc.kind == "ExternalInput":
            if name != partition_name:
                in_names.append(name)
        elif alloc.kind == "ExternalOutput":
            out_avals.append(jax.core.ShapedArray(tuple(alloc.tensor_shape),
                                                  _mb.dt.np(alloc.dtype)))
            out_names.append(name)
    n_params = len(in_names)
    n_outs = len(out_avals)
    all_in = list(in_names) + list(out_names)
    if partition_name is not None:
        all_in.append(partition_name)

    def _body(*args):
        operands = list(args)
        if partition_name is not None:
            operands.append(bass2jax.partition_id_tensor())
        return tuple(_bass_exec_p.bind(
            *operands, out_avals=tuple(out_avals), in_names=tuple(all_in),
            out_names=tuple(out_names), lowering_input_output_aliases=(),
            sim_require_finite=True, sim_require_nnan=True, nc=nc))

    devices = jax.devices()[:_B]
    mesh = Mesh(np.asarray(devices), ("core",))
    sharded = jax.jit(
        shard_map(_body, mesh=mesh,
                  in_specs=(PartitionSpec("core"),) * (n_params + n_outs),
                  out_specs=(PartitionSpec("core"),) * n_outs,
                  check_rep=False),
        keep_unused=True)
    zeros_dev = [jax.device_put(
        np.zeros((_B * a.shape[0],) + tuple(a.shape[1:]), a.dtype))
        for a in out_avals]
    _CACHE["runner"] = (sharded, in_names, out_names, out_avals, zeros_dev)
    return _CACHE["runner"]


def _pack_inputs(inputs):
    packs = []
    for b in range(_B):
        packs.append(np.concatenate(
            [np.asarray(inputs[n][b] if n in ("coords", "features") else inputs[n],
                        dtype=np.float32).ravel() for n, _ in WSHAPES]))
    return [np.concatenate(packs, axis=0)]


def run(inputs):
    sharded, in_names, out_names, out_avals, zeros_dev = _get_runner()
    assert in_names == ["packed"], in_names
    outs = sharded(*_pack_inputs(inputs), *zeros_dev)
    i = out_names.index("out")
    return np.asarray(outs[i]).reshape(_B, *out_avals[i].shape).astype(np.float32)


def time_kernel(inputs, iters=20):
    """Steady-state per-invocation wall time (ns), pipelined dispatch."""
    import time as _time
    import jax
    sharded, in_names, out_names, out_avals, zeros_dev = _get_runner()
    dev_in = [jax.device_put(a) for a in _pack_inputs(inputs)]
    for _ in range(2):  # warm
        jax.block_until_ready(sharded(*dev_in, *zeros_dev))
    t0 = _time.perf_counter()
    outs = [sharded(*dev_in, *zeros_dev) for _ in range(iters)]
    jax.block_until_ready(outs)
    return (_time.perf_counter() - t0) / iters * 1e9


def kernel(**inputs):
    return run(inputs)


# revision 8
# speedup vs baseline: 50.7528x; 1.3109x over previous
"""Bass/Tile kernel for nn_PointCloudMPE: per-core pipeline for one batch
(coords [4096,3], features [4096,64]) -> out [4096,16].

Pipeline: pairwise -dist^2 via PE matmul (homogeneous 5-dim trick) ->
top-16 threshold via DVE max8/match_replace/max8 -> 0/1 neighbor mask ->
masked moment sums via PE matmuls (mask transposed on PE) -> 3x3 covariance
-> batched branch-free cyclic Jacobi eigensolver -> normals/invariants ->
MLPs on PE (transposed activations) -> assembled [4096,16] output.
"""

import numpy as np

import concourse.bass as bass
import concourse.bacc as bacc
import concourse.mybir as mybir
import concourse.tile as tile
from concourse.masks import make_identity

f32 = mybir.dt.float32
f32r = mybir.dt.float32r
AF = mybir.ActivationFunctionType
OP = mybir.AluOpType

N, K, NB = 4096, 16, 32  # points, knn, row blocks of 128
NEG_BIG = -1.0e30

# packed input layout (flat f32 offsets)
WSHAPES = [("coords", (N, 3)), ("features", (N, 64)),
           ("inv_W1", (6, 128)), ("inv_b1", (128,)), ("inv_W2", (128, 128)),
           ("inv_b2", (128,)), ("feat_W1", (64, 128)), ("feat_b1", (128,)),
           ("feat_W2", (128, 128)), ("feat_b2", (128,)), ("sh_W1", (256, 128)),
           ("sh_b1", (128,)), ("sh_W2", (128, 128)), ("sh_b2", (128,)),
           ("g0_W", (128, 1)), ("g0_b", (1,))]
POFS = {}
_o = 0
for _n, _s in WSHAPES:
    POFS[_n] = _o
    _o += int(np.prod(_s))
PACKED_LEN = _o


def _sl(t, q):
    # [128, 32, C] tile -> [128, 32] strided view of component q
    return t[:, :, q : q + 1].squeeze(2)


def build_nc():
    nc = bacc.Bacc("TRN2", target_bir_lowering=False)

    packed_h = nc.declare_dram_parameter("packed", [PACKED_LEN], f32, isOutput=False)
    out_h = nc.declare_dram_parameter("out", [N, 16], f32, isOutput=True)

    with tile.TileContext(nc) as tc:
        _body(nc, tc, packed_h, out_h)
    nc.finalize()
    return nc


def _pk(packed_h, name, shape):
    ofs = POFS[name]
    n = int(np.prod(shape))
    ap = packed_h[ofs:ofs + n]
    if len(shape) == 1:
        return ap.rearrange("(a b) -> a b", b=1)
    assert len(shape) == 2
    return ap.rearrange("(a b) -> a b", b=shape[1])


def _body(nc, tc, packed_h, out_h):
    import contextlib

    ctx = contextlib.ExitStack()
    with ctx:
        persist = ctx.enter_context(tc.tile_pool(name="persist", bufs=1))
        psum_small = ctx.enter_context(tc.tile_pool(name="ps_small", bufs=1, space="PSUM"))

        ident = persist.tile([128, 128], f32)
        make_identity(nc, ident[:])

        # ---- load coords in block layout + weights ----
        coords_nat = persist.tile([128, NB, 3], f32)
        nc.sync.dma_start(
            coords_nat[:],
            packed_h[POFS["coords"]:POFS["coords"] + N * 3]
            .rearrange("(rb p d) -> p rb d", p=128, d=3))

        wdims = {"inv_W1": (6, 128), "inv_W2": (128, 128), "feat_W1": (64, 128),
                 "feat_W2": (128, 128), "sh_W2": (128, 128), "g0_W": (128, 1)}
        w = {}
        for k, shp in wdims.items():
            w[k] = persist.tile(list(shp), f32, name=f"w_{k}")
            nc.sync.dma_start(w[k][:], _pk(packed_h, k, shp))
        w["sh_W1a"] = persist.tile([128, 128], f32, name="w_sh_W1a")
        w["sh_W1b"] = persist.tile([128, 128], f32, name="w_sh_W1b")
        sh1 = _pk(packed_h, "sh_W1", (256, 128))
        nc.sync.dma_start(w["sh_W1a"][:], sh1[0:128, :])
        nc.sync.dma_start(w["sh_W1b"][:], sh1[128:256, :])
        for k in ("inv_b1", "inv_b2", "feat_b1", "feat_b2", "sh_b1", "sh_b2"):
            w[k] = persist.tile([128, 1], f32, name=f"w_{k}")
            nc.sync.dma_start(w[k][:], _pk(packed_h, k, (128,)))
        g0b_one = persist.tile([1, 1], f32)
        nc.sync.dma_start(g0b_one[:], _pk(packed_h, "g0_b", (1,)))
        ones_row = persist.tile([1, 128], f32)
        nc.vector.memset(ones_row[:], 1.0)
        w["g0_b"] = persist.tile([128, 1], f32, name="w_g0_b_bc")
        g0ps = psum_small.tile([128, 1], f32, name="tp")
        nc.tensor.matmul(g0ps[:], ones_row[:], g0b_one[:], start=True, stop=True)
        nc.scalar.activation(w["g0_b"][:], g0ps[:], AF.Copy)

        # ---- phi [128, 32, 9]: x,y,z,xx,xy,xz,yy,yz,zz per point ----
        phi = persist.tile([128, NB, 9], f32)
        nc.vector.tensor_copy(phi[:, :, 0:3], coords_nat[:])
        pq = [(0, 0), (0, 1), (0, 2), (1, 1), (1, 2), (2, 2)]
        for qi, (a, b) in enumerate(pq):
            nc.vector.tensor_tensor(
                phi[:, :, 3 + qi : 4 + qi], coords_nat[:, :, a : a + 1],
                coords_nat[:, :, b : b + 1], OP.mult)

        # ---- L/R matrices for -dist^2 matmul, via block-layout staging ----
        # L col j: (x,y,z,sq,1); R col j: (2x,2y,2z,-1,-sq)
        sq_nat = persist.tile([128, NB], f32)
        nc.vector.tensor_add(sq_nat[:], _sl(phi, 3), _sl(phi, 6))
        nc.vector.tensor_add(sq_nat[:], sq_nat[:], _sl(phi, 8))
        Lstage = persist.tile([128, NB, 5], f32)
        Rstage = persist.tile([128, NB, 5], f32)
        nc.vector.tensor_copy(Lstage[:, :, 0:3], coords_nat[:])
        nc.vector.tensor_copy(_sl(Lstage, 3), sq_nat[:])
        nc.vector.memset(_sl(Lstage, 4), 1.0)
        nc.scalar.activation(Rstage[:, :, 0:3], coords_nat[:], AF.Copy, scale=2.0)
        nc.vector.memset(_sl(Rstage, 3), -1.0)
        nc.scalar.activation(_sl(Rstage, 4), sq_nat[:], AF.Copy, scale=-1.0)

        L_all = persist.tile([5, N], f32)
        R_all = persist.tile([5, N], f32)
        for rb in range(NB):
            tpl = psum_small.tile([5, 128], f32, name="tp")
            nc.tensor.transpose(tpl[:], Lstage[:, rb : rb + 1, :].squeeze(1), ident[:])
            nc.scalar.activation(L_all[:, rb * 128:(rb + 1) * 128], tpl[:], AF.Copy)
            tpr = psum_small.tile([5, 128], f32, name="tp")
            nc.tensor.transpose(tpr[:], Rstage[:, rb : rb + 1, :].squeeze(1), ident[:])
            nc.scalar.activation(R_all[:, rb * 128:(rb + 1) * 128], tpr[:], AF.Copy)

        # ---- featT [64, 4096] ----
        featT = persist.tile([64, N], f32)
        with tc.tile_pool(name="ftile", bufs=2) as fpool:
            for rb in range(NB):
                ft = fpool.tile([128, 64], f32)
                nc.sync.dma_start(
                    ft[:],
                    packed_h[POFS["features"] + rb * 128 * 64:
                             POFS["features"] + (rb + 1) * 128 * 64]
                    .rearrange("(p d) -> p d", d=64))
                tp = psum_small.tile([64, 128], f32, name="tp")
                nc.tensor.transpose(tp[:], ft[:], ident[:])
                nc.scalar.activation(featT[:, rb * 128:(rb + 1) * 128], tp[:], AF.Copy)

        # ---- per-point accumulators (block layout [128, 32]) ----
        radii = persist.tile([128, NB], f32)
        Sall = persist.tile([128, 9, NB], f32)

        # ---- main selection + moment loop over row blocks ----
        with (
            tc.tile_pool(name="ndpool", bufs=2) as ndpool,
            tc.tile_pool(name="scrpool", bufs=2) as scrpool,
            tc.tile_pool(name="mpool", bufs=2) as mpool,
            tc.tile_pool(name="mtpool", bufs=2) as mtpool,
            tc.tile_pool(name="selpool", bufs=2) as selpool,
            tc.tile_pool(name="ps_nd", bufs=2, space="PSUM") as ps_nd,
            tc.tile_pool(name="ps_mt", bufs=2, space="PSUM") as ps_mt,
            tc.tile_pool(name="ps_s", bufs=1, space="PSUM") as ps_s,
        ):
            def compute_nd(rb):
                nd = ndpool.tile([128, N], f32, name="nd")
                lhs = L_all[:, rb * 128:(rb + 1) * 128]
                for half in range(4):
                    ps = ps_nd.tile([128, 1024], f32, name="ps")
                    for cc in range(2):
                        c0 = (2 * half + cc) * 512
                        nc.tensor.matmul(ps[:, cc * 512:(cc + 1) * 512], lhs,
                                         R_all[:, c0:c0 + 512], start=True, stop=True)
                    nc.scalar.activation(nd[:, half * 1024:(half + 1) * 1024],
                                         ps[:], AF.Copy)
                # self-exclusion: diagonal block columns rb*128..rb*128+127
                nc.gpsimd.affine_select(
                    out=nd[:, rb * 128:(rb + 1) * 128],
                    in_=nd[:, rb * 128:(rb + 1) * 128],
                    compare_op=OP.not_equal, fill=NEG_BIG,
                    base=0, pattern=[[1, 128]], channel_multiplier=-1)
                return nd

            nd_next = compute_nd(0)
            for rb in range(NB):
                nd = nd_next
                if rb + 1 < NB:
                    nd_next = compute_nd(rb + 1)

                mx1 = selpool.tile([128, 8], f32)
                mx2 = selpool.tile([128, 8], f32)
                scr = scrpool.tile([128, N], f32)
                nc.vector.max(out=mx1[:], in_=nd[:])
                nc.vector.match_replace(out=scr[:], in_to_replace=mx1[:],
                                        in_values=nd[:], imm_value=NEG_BIG)
                nc.vector.max(out=mx2[:], in_=scr[:])

                mask = mpool.tile([128, N], f32)
                nc.gpsimd.tensor_scalar(mask[:], nd[:], mx2[:, 7:8], None, op0=OP.is_ge)

                # radius: the 16 selected -dist^2 values are mx1/mx2 [128,8];
                # sum sqrt(dist) via two tiny clamped sqrts with accumulate
                mc1 = selpool.tile([128, 8], f32, name="mc1")
                mc2 = selpool.tile([128, 8], f32, name="mc2")
                nc.vector.tensor_scalar(mc1[:], mx1[:], 0.0, None, op0=OP.min)
                nc.vector.tensor_scalar(mc2[:], mx2[:], 0.0, None, op0=OP.min)
                r1 = selpool.tile([128, 1], f32, name="r1")
                r2 = selpool.tile([128, 1], f32, name="r2")
                nc.scalar.activation(mc1[:], mc1[:], AF.Sqrt, scale=-1.0,
                                     accum_out=r1[:])
                nc.scalar.activation(mc2[:], mc2[:], AF.Sqrt, scale=-1.0,
                                     accum_out=r2[:])
                nc.vector.tensor_add(radii[:, rb : rb + 1], r1[:], r2[:])

                # transpose mask strip -> MT [j, i] for this row block
                mt = mtpool.tile([128, N], f32)
                for g in range(8):
                    tp = ps_mt.tile([128, 512], f32)
                    for kk in range(4):
                        c0 = (4 * g + kk) * 128
                        nc.tensor.transpose(tp[:, kk * 128:(kk + 1) * 128],
                                            mask[:, c0:c0 + 128], ident[:])
                    nc.scalar.activation(mt[:, g * 512:(g + 1) * 512], tp[:], AF.Copy)

                sps = ps_s.tile([128, 9], f32)
                for jb in range(NB):
                    nc.tensor.matmul(sps[:], mt[:, jb * 128:(jb + 1) * 128],
                                     phi[:, jb : jb + 1, :].squeeze(1), start=(jb == 0), stop=(jb == NB - 1))
                nc.scalar.activation(Sall[:, :, rb : rb + 1].squeeze(2), sps[:], AF.Copy)

        # ================= covariance + Jacobi + invariants =================
        jp = ctx.enter_context(tc.tile_pool(name="jacobi", bufs=1))

        _jtc = [0]

        def jt():
            _jtc[0] += 1
            return jp.tile([128, NB], f32, name=f"jt{_jtc[0]}")

        cx, cy, cz = (_sl(coords_nat, q) for q in range(3))
        inv_k = 1.0 / K

        # neighbor means
        m1 = [jp.tile([128, NB], f32, name=f"m1_{q}") for q in range(3)]
        for q in range(3):
            nc.scalar.activation(m1[q][:], Sall[:, q : q + 1, :].squeeze(1), AF.Copy, scale=inv_k)

        # cov components: S2ab/K - ca*m1b - cb*m1a + ca*cb
        cvs = [cx, cy, cz]
        A = {}
        for qi, (a, b) in enumerate(pq):
            t0 = jt()
            nc.scalar.activation(t0[:], Sall[:, 3 + qi : 4 + qi, :].squeeze(1), AF.Copy, scale=inv_k)
            t1 = jt()
            nc.vector.tensor_tensor(t1[:], cvs[a], m1[b][:], OP.mult)
            nc.vector.tensor_sub(t0[:], t0[:], t1[:])
            nc.vector.tensor_tensor(t1[:], cvs[b], m1[a][:], OP.mult)
            nc.vector.tensor_sub(t0[:], t0[:], t1[:])
            nc.vector.tensor_tensor(t1[:], cvs[a], cvs[b], OP.mult)
            nc.vector.tensor_add(t0[:], t0[:], t1[:])
            A[(a, b)] = t0

        # V = I (9 tiles)
        V = {}
        for i in range(3):
            for j in range(3):
                V[(i, j)] = jp.tile([128, NB], f32, name=f"V{i}{j}")
                nc.vector.memset(V[(i, j)][:], 1.0 if i == j else 0.0)

        def Ag(i, j):
            return A[(i, j)] if i <= j else A[(j, i)]

        tmp1, tmp2, tmp3, tmp4 = jt(), jt(), jt(), jt()
        SWEEPS = 3
        for sw in range(SWEEPS):
            for (p, q) in ((0, 1), (0, 2), (1, 2)):
                app, aqq, apq = Ag(p, p), Ag(q, q), Ag(p, q)
                # theta = (aqq - app) / (2*apq), guarded; t = sgn/( |th| + sqrt(th^2+1) )
                sA = tmp1
                nc.vector.tensor_scalar(sA[:], apq[:], 0.0, None, op0=OP.is_ge)
                nc.vector.tensor_scalar(sA[:], sA[:], 2.0, -1.0, op0=OP.mult, op1=OP.add)
                den = tmp2
                nc.vector.tensor_tensor(den[:], apq[:], sA[:], OP.mult)  # |apq|
                nc.vector.tensor_scalar(den[:], den[:], 1e-30, 2.0, op0=OP.max, op1=OP.mult)
                nc.vector.reciprocal(den[:], den[:])                     # 1/(2|apq|)
                th = tmp3
                nc.vector.tensor_sub(th[:], app[:], aqq[:])
                nc.vector.tensor_tensor(th[:], th[:], den[:], OP.mult)
                nc.vector.tensor_tensor(th[:], th[:], sA[:], OP.mult)    # theta (signed)
                nc.vector.tensor_scalar(th[:], th[:], 1e17, -1e17, op0=OP.min, op1=OP.max)
                sT = tmp1
                nc.vector.tensor_scalar(sT[:], th[:], 0.0, None, op0=OP.is_ge)
                nc.vector.tensor_scalar(sT[:], sT[:], 2.0, -1.0, op0=OP.mult, op1=OP.add)
                tha = tmp2
                nc.vector.tensor_tensor(tha[:], th[:], sT[:], OP.mult)   # |theta|
                r_ = tmp4
                nc.vector.tensor_tensor(r_[:], th[:], th[:], OP.mult)
                nc.scalar.activation(r_[:], r_[:], AF.Sqrt, bias=1.0)    # sqrt(th^2+1)
                nc.vector.tensor_add(r_[:], r_[:], tha[:])
                nc.vector.reciprocal(r_[:], r_[:])
                tt = tmp2
                nc.vector.tensor_tensor(tt[:], r_[:], sT[:], OP.mult)    # tan(phi)
                cc_ = tmp3
                nc.vector.tensor_tensor(cc_[:], tt[:], tt[:], OP.mult)
                nc.scalar.activation(cc_[:], cc_[:], AF.Sqrt, bias=1.0)
                nc.vector.reciprocal(cc_[:], cc_[:])                     # cos(phi)
                ss_ = tmp4
                nc.vector.tensor_tensor(ss_[:], tt[:], cc_[:], OP.mult)  # sin(phi)
                cs, sn = cc_, ss_

                # update A
                c2, s2, csn = jt(), jt(), jt()
                nc.vector.tensor_tensor(c2[:], cs[:], cs[:], OP.mult)
                nc.vector.tensor_tensor(s2[:], sn[:], sn[:], OP.mult)
                nc.vector.tensor_tensor(csn[:], cs[:], sn[:], OP.mult)
                u1, u2, u3, u4 = jt(), jt(), jt(), jt()
                nc.vector.tensor_tensor(u1[:], c2[:], app[:], OP.mult)
                nc.vector.tensor_tensor(u2[:], csn[:], apq[:], OP.mult)
                nc.vector.tensor_tensor(u3[:], s2[:], aqq[:], OP.mult)
                nc.vector.tensor_tensor(u4[:], s2[:], app[:], OP.mult)
                nc.gpsimd.tensor_tensor(app[:], c2[:], aqq[:], OP.mult)  # app <- c2*aqq (tmp)
                # app' = u1 + 2*u2 + u3 ; aqq' = u4 - 2*u2 + c2*aqq
                nc.vector.scalar_tensor_tensor(u1[:], u2[:], 2.0, u1[:], op0=OP.mult, op1=OP.add)
                nc.vector.tensor_add(u1[:], u1[:], u3[:])
                nc.vector.scalar_tensor_tensor(u4[:], u2[:], -2.0, u4[:], op0=OP.mult, op1=OP.add)
                nc.vector.tensor_add(aqq[:], u4[:], app[:])
                nc.vector.tensor_copy(app[:], u1[:])
                nc.vector.memset(apq[:], 0.0)
                rr = 3 - p - q
                apr, aqr = Ag(p, rr), Ag(q, rr)
                nc.vector.tensor_tensor(u1[:], cs[:], apr[:], OP.mult)
                nc.vector.tensor_tensor(u2[:], sn[:], aqr[:], OP.mult)
                nc.vector.tensor_tensor(u3[:], sn[:], apr[:], OP.mult)
                nc.vector.tensor_tensor(u4[:], cs[:], aqr[:], OP.mult)
                nc.vector.tensor_add(apr[:], u1[:], u2[:])
                nc.vector.tensor_sub(aqr[:], u4[:], u3[:])
                # update V columns p,q (rows 0..2) on gpsimd
                for i in range(3):
                    vip, viq = V[(i, p)], V[(i, q)]
                    nc.gpsimd.tensor_tensor(u1[:], cs[:], vip[:], OP.mult)
                    nc.gpsimd.tensor_tensor(u2[:], sn[:], viq[:], OP.mult)
                    nc.gpsimd.tensor_tensor(u3[:], sn[:], vip[:], OP.mult)
                    nc.gpsimd.tensor_tensor(u4[:], cs[:], viq[:], OP.mult)
                    nc.gpsimd.tensor_add(vip[:], u1[:], u2[:])
                    nc.gpsimd.tensor_sub(viq[:], u4[:], u3[:])

        e0, e1, e2 = A[(0, 0)], A[(1, 1)], A[(2, 2)]
        smin, smid, smax = jt(), jt(), jt()
        nc.vector.tensor_tensor(smin[:], e0[:], e1[:], OP.min)
        nc.vector.tensor_tensor(smin[:], smin[:], e2[:], OP.min)
        nc.vector.tensor_tensor(smax[:], e0[:], e1[:], OP.max)
        nc.vector.tensor_tensor(smax[:], smax[:], e2[:], OP.max)
        nc.vector.tensor_add(smid[:], e0[:], e1[:])
        nc.vector.tensor_add(smid[:], smid[:], e2[:])
        esum = jt()
        nc.vector.tensor_copy(esum[:], smid[:])  # e0+e1+e2
        nc.vector.tensor_sub(smid[:], smid[:], smin[:])
        nc.vector.tensor_sub(smid[:], smid[:], smax[:])

        # select eigenvector of smallest eigenvalue: exclusive masks
        m0, m1m, m2m = jt(), jt(), jt()
        nc.vector.tensor_tensor(tmp1[:], e0[:], e1[:], OP.is_le)
        nc.vector.tensor_tensor(tmp2[:], e0[:], e2[:], OP.is_le)
        nc.vector.tensor_tensor(m0[:], tmp1[:], tmp2[:], OP.mult)
        nc.vector.tensor_tensor(tmp1[:], e1[:], e0[:], OP.is_lt)
        nc.vector.tensor_tensor(tmp2[:], e1[:], e2[:], OP.is_le)
        nc.vector.tensor_tensor(m1m[:], tmp1[:], tmp2[:], OP.mult)
        nc.vector.tensor_add(m2m[:], m0[:], m1m[:])
        nc.vector.tensor_scalar(m2m[:], m2m[:], -1.0, 1.0, op0=OP.mult, op1=OP.add)

        nrm = [jp.tile([128, NB], f32, name=f"nrm{q}") for q in range(3)]  # normals nx,ny,nz
        for i in range(3):
            nc.vector.tensor_tensor(tmp1[:], m0[:], V[(i, 0)][:], OP.mult)
            nc.vector.tensor_tensor(tmp2[:], m1m[:], V[(i, 1)][:], OP.mult)
            nc.vector.tensor_add(tmp1[:], tmp1[:], tmp2[:])
            nc.vector.tensor_tensor(tmp2[:], m2m[:], V[(i, 2)][:], OP.mult)
            nc.vector.tensor_add(nrm[i][:], tmp1[:], tmp2[:])

        # center + outward + orientation
        ones128 = jp.tile([128, 1], f32)
        nc.vector.memset(ones128[:], 1.0)
        partials = jp.tile([128, 3], f32)
        for q, cv in enumerate(cvs):
            nc.vector.tensor_reduce(partials[:, q : q + 1], cv, axis=mybir.AxisListType.X,
                                    op=OP.add)
        cps = psum_small.tile([1, 3], f32, name="tp")
        nc.tensor.matmul(cps[:], ones128[:], partials[:], start=True, stop=True)
        center1 = jp.tile([1, 3], f32)
        nc.scalar.activation(center1[:], cps[:], AF.Copy, scale=1.0 / N)
        center = jp.tile([128, 3], f32)
        cbps = psum_small.tile([128, 3], f32, name="tp")
        nc.tensor.matmul(cbps[:], ones_row[:], center1[:], start=True, stop=True)
        nc.scalar.activation(center[:], cbps[:], AF.Copy)

        ox, oy, oz = jt(), jt(), jt()
        ovs = [ox, oy, oz]
        for q in range(3):
            nc.vector.tensor_scalar(ovs[q][:], cvs[q], center[:, q : q + 1], None,
                                    op0=OP.subtract)
        dt_ = jt()
        nc.vector.tensor_tensor(dt_[:], nrm[0][:], ox[:], OP.mult)
        nc.vector.tensor_tensor(tmp1[:], nrm[1][:], oy[:], OP.mult)
        nc.vector.tensor_add(dt_[:], dt_[:], tmp1[:])
        nc.vector.tensor_tensor(tmp1[:], nrm[2][:], oz[:], OP.mult)
        nc.vector.tensor_add(dt_[:], dt_[:], tmp1[:])
        nc.vector.tensor_scalar(dt_[:], dt_[:], 0.0, None, op0=OP.is_ge)
        nc.vector.tensor_scalar(dt_[:], dt_[:], 2.0, -1.0, op0=OP.mult, op1=OP.add)
        for i in range(3):
            nc.vector.tensor_tensor(nrm[i][:], nrm[i][:], dt_[:], OP.mult)
        # normalize
        nc.vector.tensor_tensor(tmp1[:], nrm[0][:], nrm[0][:], OP.mult)
        nc.vector.tensor_tensor(tmp2[:], nrm[1][:], nrm[1][:], OP.mult)
        nc.vector.tensor_add(tmp1[:], tmp1[:], tmp2[:])
        nc.vector.tensor_tensor(tmp2[:], nrm[2][:], nrm[2][:], OP.mult)
        nc.vector.tensor_add(tmp1[:], tmp1[:], tmp2[:])
        nc.scalar.activation(tmp1[:], tmp1[:], AF.Sqrt)
        nc.vector.tensor_scalar(tmp1[:], tmp1[:], 1e-6, None, op0=OP.max)
        nc.vector.reciprocal(tmp1[:], tmp1[:])
        for i in range(3):
            nc.vector.tensor_tensor(nrm[i][:], nrm[i][:], tmp1[:], OP.mult)

        # centered radius + dominance + plane offset
        cr = jt()
        nc.vector.tensor_tensor(cr[:], ox[:], ox[:], OP.mult)
        nc.vector.tensor_tensor(tmp1[:], oy[:], oy[:], OP.mult)
        nc.vector.tensor_add(cr[:], cr[:], tmp1[:])
        nc.vector.tensor_tensor(tmp1[:], oz[:], oz[:], OP.mult)
        nc.vector.tensor_add(cr[:], cr[:], tmp1[:])
        nc.scalar.activation(cr[:], cr[:], AF.Sqrt)
        dom = jt()
        nc.vector.tensor_scalar(tmp1[:], esum[:], 1e-6, None, op0=OP.max)
        nc.vector.reciprocal(tmp1[:], tmp1[:])
        nc.vector.tensor_tensor(dom[:], smax[:], tmp1[:], OP.mult)
        pd = jt()  # + sum(c*n); negated at assembly
        nc.vector.tensor_tensor(pd[:], cx, nrm[0][:], OP.mult)
        nc.vector.tensor_tensor(tmp1[:], cy, nrm[1][:], OP.mult)
        nc.vector.tensor_add(pd[:], pd[:], tmp1[:])
        nc.vector.tensor_tensor(tmp1[:], cz, nrm[2][:], OP.mult)
        nc.vector.tensor_add(pd[:], pd[:], tmp1[:])

        # ================= MLPs (transposed activations) =================
        inv_stage = jp.tile([128, NB, 6], f32)
        nc.vector.tensor_copy(_sl(inv_stage, 0), smin[:])
        nc.vector.tensor_copy(_sl(inv_stage, 1), smid[:])
        nc.vector.tensor_copy(_sl(inv_stage, 2), smax[:])
        nc.scalar.activation(_sl(inv_stage, 3), radii[:], AF.Copy, scale=1.0 / K)
        nc.vector.tensor_copy(_sl(inv_stage, 4), cr[:])
        nc.vector.tensor_copy(_sl(inv_stage, 5), dom[:])

        invT = jp.tile([6, N], f32)
        for rb in range(NB):
            tp = psum_small.tile([6, 128], f32, name="tp")
            nc.tensor.transpose(tp[:], inv_stage[:, rb : rb + 1, :].squeeze(1), ident[:])
            nc.scalar.activation(invT[:, rb * 128:(rb + 1) * 128], tp[:], AF.Copy)

        hidden = jp.tile([128, N], f32)
        outstage = jp.tile([128, NB, 16], f32)
        nc.vector.memset(outstage[:], 0.0)

        with (
            tc.tile_pool(name="mlp_sb", bufs=3) as mlp_sb,
            tc.tile_pool(name="ps_mlp", bufs=4, space="PSUM") as ps_mlp,
        ):
            for ch in range(8):
                cslice = slice(ch * 512, (ch + 1) * 512)
                # inv branch
                p1 = ps_mlp.tile([128, 512], f32, name="pmlp")
                nc.tensor.matmul(p1[:], w["inv_W1"][:],
                                 invT[:, cslice], start=True, stop=True)
                h1 = mlp_sb.tile([128, 512], f32)
                nc.scalar.activation(h1[:], p1[:], AF.Gelu, bias=w["inv_b1"][:])
                p2 = ps_mlp.tile([128, 512], f32, name="pmlp")
                nc.tensor.matmul(p2[:], w["inv_W2"][:],
                                 h1[:], start=True, stop=True)
                ih = mlp_sb.tile([128, 512], f32)
                nc.scalar.activation(ih[:], p2[:], AF.Identity, bias=w["inv_b2"][:])
                # feat branch
                p3 = ps_mlp.tile([128, 512], f32, name="pmlp")
                nc.tensor.matmul(p3[:], w["feat_W1"][:],
                                 featT[:, cslice], start=True, stop=True)
                h2 = mlp_sb.tile([128, 512], f32)
                nc.scalar.activation(h2[:], p3[:], AF.Gelu, bias=w["feat_b1"][:])
                p4 = ps_mlp.tile([128, 512], f32, name="pmlp")
                nc.tensor.matmul(p4[:], w["feat_W2"][:],
                                 h2[:], start=True, stop=True)
                fh = mlp_sb.tile([128, 512], f32)
                nc.scalar.activation(fh[:], p4[:], AF.Identity, bias=w["feat_b2"][:])
                # shared
                p5 = ps_mlp.tile([128, 512], f32, name="pmlp")
                nc.tensor.matmul(p5[:], w["sh_W1a"][:], ih[:],
                                 start=True, stop=False)
                nc.tensor.matmul(p5[:], w["sh_W1b"][:], fh[:],
                                 start=False, stop=True)
                hs = mlp_sb.tile([128, 512], f32)
                nc.scalar.activation(hs[:], p5[:], AF.Gelu, bias=w["sh_b1"][:])
                p6 = ps_mlp.tile([128, 512], f32, name="pmlp")
                nc.tensor.matmul(p6[:], w["sh_W2"][:], hs[:],
                                 start=True, stop=True)
                nc.scalar.activation(hidden[:, cslice], p6[:], AF.Identity,
                                     bias=w["sh_b2"][:])

            # scalar head: per row block, lhsT = hidden strip, rhs = g0_W
            for rb in range(NB):
                sp = ps_mlp.tile([128, 1], f32, name="psp", bufs=2)
                nc.tensor.matmul(sp[:], hidden[:, rb * 128:(rb + 1) * 128], w["g0_W"][:],
                                 start=True, stop=True)
                nc.scalar.activation(outstage[:, rb : rb + 1, 0:1].squeeze(1), sp[:], AF.Identity,
                                     bias=w["g0_b"][:])

        # ================= assembly + output DMA =================
        nc.vector.tensor_copy(_sl(outstage, 1), nrm[0][:])
        nc.vector.tensor_copy(_sl(outstage, 2), nrm[1][:])
        nc.scalar.activation(_sl(outstage, 3), nrm[2][:], AF.Copy, scale=-1.0)
        nc.scalar.activation(_sl(outstage, 4), pd[:], AF.Copy, scale=-1.0)
        nc.vector.tensor_copy(_sl(outstage, 5), nrm[0][:])
        nc.vector.tensor_copy(_sl(outstage, 6), nrm[1][:])
        nc.vector.tensor_copy(_sl(outstage, 7), nrm[2][:])
        nc.vector.tensor_copy(_sl(outstage, 11), cx)
        nc.vector.tensor_copy(_sl(outstage, 12), cy)
        nc.vector.tensor_copy(_sl(outstage, 13), cz)
        nc.vector.memset(_sl(outstage, 14), 1.0)

        for rb in range(NB):
            nc.sync.dma_start(out_h[rb * 128:(rb + 1) * 128, :], outstage[:, rb : rb + 1, :].squeeze(1))


# ---------------------------------------------------------------------------
# SPMD driver: batch b -> core b (persistent PJRT executable, packed input)
# ---------------------------------------------------------------------------
_B = 8
_CACHE = {}


def _get_runner():
    if "runner" in _CACHE:
        return _CACHE["runner"]
    import jax
    from jax.sharding import Mesh, PartitionSpec
    from jax.experimental.shard_map import shard_map
    from concourse import bass2jax, mybir as _mb
    from concourse.bass2jax import _bass_exec_p, install_neuronx_cc_hook

    install_neuronx_cc_hook()
    nc = build_nc()
    partition_name = nc.partition_id_tensor.name if nc.partition_id_tensor else None
    in_names, out_names, out_avals = [], [], []
    for alloc in nc.m.functions[0].allocations:
        if not isinstance(alloc, _mb.MemoryLocationSet):
            continue
        name = alloc.memorylocations[0].name
        if alloc.kind == "ExternalInput":
            if name != partition_name:
                in_names.append(name)
        elif alloc.kind == "ExternalOutput":
            out_avals.append(jax.core.ShapedArray(tuple(alloc.tensor_shape),
                                                  _mb.dt.np(alloc.dtype)))
            out_names.append(name)
    n_params = len(in_names)
    n_outs = len(out_avals)
    all_in = list(in_names) + list(out_names)
    if partition_name is not None:
        all_in.append(partition_name)

    def _body(*args):
        operands = list(args)
        if partition_name is not None:
            operands.append(bass2jax.partition_id_tensor())
        return tuple(_bass_exec_p.bind(
            *operands, out_avals=tuple(out_avals), in_names=tuple(all_in),
            out_names=tuple(out_names), lowering_input_output_aliases=(),
            sim_require_finite=True, sim_require_nnan=True, nc=nc))

    devices = jax.devices()[:_B]
    mesh = Mesh(np.asarray(devices), ("core",))
    sharded = jax.jit(
        shard_map(_body, mesh=mesh,
                  in_specs=(PartitionSpec("core"),) * (n_params + n_outs),
                  out_specs=(PartitionSpec("core"),) * n_outs,
                  check_rep=False),
        keep_unused=True)
    zeros_dev = [jax.device_put(
        np.zeros((_B * a.shape[0],) + tuple(a.shape[1:]), a.dtype))
        for a in out_avals]
    _CACHE["runner"] = (sharded, in_names, out_names, out_avals, zeros_dev)
    return _CACHE["runner"]


def _pack_inputs(inputs):
    packs = []
    for b in range(_B):
        packs.append(np.concatenate(
            [np.asarray(inputs[n][b] if n in ("coords", "features") else inputs[n],
                        dtype=np.float32).ravel() for n, _ in WSHAPES]))
    return [np.concatenate(packs, axis=0)]


def run(inputs):
    sharded, in_names, out_names, out_avals, zeros_dev = _get_runner()
    assert in_names == ["packed"], in_names
    outs = sharded(*_pack_inputs(inputs), *zeros_dev)
    i = out_names.index("out")
    return np.asarray(outs[i]).reshape(_B, *out_avals[i].shape).astype(np.float32)


def time_kernel(inputs, iters=20):
    """Steady-state per-invocation wall time (ns), pipelined dispatch."""
    import time as _time
    import jax
    sharded, in_names, out_names, out_avals, zeros_dev = _get_runner()
    dev_in = [jax.device_put(a) for a in _pack_inputs(inputs)]
    for _ in range(2):  # warm
        jax.block_until_ready(sharded(*dev_in, *zeros_dev))
    t0 = _time.perf_counter()
    outs = [sharded(*dev_in, *zeros_dev) for _ in range(iters)]
    jax.block_until_ready(outs)
    return (_time.perf_counter() - t0) / iters * 1e9


def kernel(**inputs):
    return run(inputs)
